# revision 1
# baseline (speedup 1.0000x reference)
"""Distance-correlation loss kernel for trn2 (8 NeuronCores, SPMD).

Math (reference): for F in {X, Y}: a = sqrt(relu(sq_i + sq_j - 2 F F^T) + eps),
row_j = colsum_j / (n-2), tot = sum / ((n-1)(n-2)), A = a - 2*row_j + tot with
zero diagonal; loss = -g_xy / sqrt(g_xx * g_yy + eps), g_PQ = sum(P*Q)/(n(n-3)).

Device strategy per core c (rows 512c..512c+512 of the distance matrix, but
computed TRANSPOSED: tiles aT[j_chunk=128, i=512]):
  pass 1: for each 128-wide j-chunk: 16 accumulating bf16 matmuls
          (stationary = xT strip [128,128], moving = core's xTc [128,512]),
          then ACT(-2*psum + sq_j bias) -> DVE(+sq_i bcast) -> DVE(relu)
          -> ACT(sqrt + eps, accum_out=per-partition colsum) -> ACT(copy -64 -> fp16 cache)
  AllReduce of [2,128,32] partial colsums; rv_shift = -2/(n-2)*C + tot + 64
  pass 2: At = ACT(cache_f16 + rv_shift bias); 3 fused tensor_tensor_reduce ops
          accumulate per-lane partials of sum(At*Bt), sum(At^2), sum(Bt^2).
Host: f64 combine of per-lane partials + bit-exact diagonal removal.
"""

import sys

for _p in ("/opt/trn_rl_repo",):
    if _p not in sys.path:
        sys.path.insert(0, _p)

import numpy as np
import ml_dtypes

import concourse.bass as bass
from concourse import bacc
import concourse.mybir as mybir
import concourse.tile as tile
from concourse.bass_utils import run_bass_kernel_spmd

N = 4096
D = 2048
NCORES = 8
ROWS = N // NCORES          # 512 distance-matrix rows per core (free dim i)
NJ = N // 128               # 32 j-chunks (partition dim of transposed tiles)
NK = D // 128               # 16 contraction chunks
EPS = 1e-18
F32 = mybir.dt.float32
BF16 = mybir.dt.bfloat16
F16 = mybir.dt.float16
AF = mybir.ActivationFunctionType
ALU = mybir.AluOpType

_CACHE = {}


def _build_nc():
    nc = bacc.Bacc(None, num_devices=NCORES, target_bir_lowering=False)

    # ---- I/O ----
    xT = nc.declare_dram_parameter("xT", [D, N], BF16, isOutput=False)
    yT = nc.declare_dram_parameter("yT", [D, N], BF16, isOutput=False)
    xTc = nc.declare_dram_parameter("xTc", [D, ROWS], BF16, isOutput=False)
    yTc = nc.declare_dram_parameter("yTc", [D, ROWS], BF16, isOutput=False)
    # sq[j] reshaped so element (p, nj) = sq[128*nj + p]  (global, same all cores)
    sqjx = nc.declare_dram_parameter("sqjx", [128, NJ], F32, isOutput=False)
    sqjy = nc.declare_dram_parameter("sqjy", [128, NJ], F32, isOutput=False)
    # per-core sq slice for the free axis (rows of this core)
    sqix = nc.declare_dram_parameter("sqix", [1, ROWS], F32, isOutput=False)
    sqiy = nc.declare_dram_parameter("sqiy", [1, ROWS], F32, isOutput=False)

    axh = nc.declare_dram_parameter("axh", [N, ROWS], F16, isOutput=True)
    ayh = nc.declare_dram_parameter("ayh", [N, ROWS], F16, isOutput=True)
    rvs = nc.declare_dram_parameter("rvs", [2, 128, NJ], F32, isOutput=True)
    pp = nc.declare_dram_parameter("pp", [128, 4], F32, isOutput=True)

    with tile.TileContext(nc) as tc:
        import contextlib

        with contextlib.ExitStack() as ctx:
            singles = ctx.enter_context(tc.tile_pool(name="singles", bufs=1))
            strips = ctx.enter_context(tc.tile_pool(name="strips", bufs=8))
            psum = ctx.enter_context(tc.tile_pool(name="psum", bufs=4, space="PSUM"))
            psum1 = ctx.enter_context(tc.tile_pool(name="psum1", bufs=1, space="PSUM"))
            temps = ctx.enter_context(tc.tile_pool(name="temps", bufs=3))
            dram = ctx.enter_context(tc.tile_pool(name="dram", bufs=1, space="DRAM"))

            # ---- residents ----
            def load_resident(name, src, shape, dtype, src_ap=None):
                t = singles.tile(shape, dtype, name=name)
                nc.sync.dma_start(out=t[:], in_=src if src_ap is None else src_ap)
                return t

            xTc_sb = singles.tile([128, NK, ROWS], BF16, name="xTc_sb")
            nc.gpsimd.dma_start(
                out=xTc_sb[:], in_=xTc[:, :].rearrange("(k p) i -> p k i", p=128)
            )
            yTc_sb = singles.tile([128, NK, ROWS], BF16, name="yTc_sb")
            nc.gpsimd.dma_start(
                out=yTc_sb[:], in_=yTc[:, :].rearrange("(k p) i -> p k i", p=128)
            )
            sqjx_sb = singles.tile([128, NJ], F32, name="sqjx_sb")
            nc.gpsimd.dma_start(out=sqjx_sb[:], in_=sqjx[:, :])
            sqjy_sb = singles.tile([128, NJ], F32, name="sqjy_sb")
            nc.gpsimd.dma_start(out=sqjy_sb[:], in_=sqjy[:, :])

            def bcast_load(name, src):
                t = singles.tile([128, ROWS], F32, name=name)
                src_b = bass.AP(
                    tensor=src[:, :].tensor,
                    offset=src[:, :].offset,
                    ap=[[0, 128], [1, ROWS]],
                )
                nc.gpsimd.dma_start(out=t[:], in_=src_b)
                return t

            sqix_sb = bcast_load("sqix_sb", sqix)
            sqiy_sb = bcast_load("sqiy_sb", sqiy)

            # const tiles built by DVE reads of the DMA'd residents: absorbs the
            # DMA-completion waits into these ops so later TS/AC instructions
            # carry at most one sync wait (hardware wait-slot limit).
            eps_sb = singles.tile([128, 1], F32, name="eps_sb")
            nc.vector.tensor_scalar(
                eps_sb[:], sqjx_sb[:, 0:1], 0.0, EPS, op0=ALU.mult, op1=ALU.add
            )
            c64_sb = singles.tile([128, 1], F32, name="c64_sb")
            nc.vector.tensor_scalar(
                c64_sb[:], sqjy_sb[:, 0:1], 0.0, 64.0, op0=ALU.mult, op1=ALU.add
            )
            ones_sb = singles.tile([128, 1], F32, name="ones_sb")
            nc.vector.tensor_scalar(
                ones_sb[:], sqix_sb[:, 0:1], 0.0, 1.0, op0=ALU.mult, op1=ALU.add
            )
            acc = singles.tile([128, 4], F32, name="acc")
            nc.vector.tensor_scalar(
                acc[:], sqiy_sb[:, 0:4], 0.0, 0.0, op0=ALU.mult, op1=ALU.add
            )

            cache_x = singles.tile([128, NJ * ROWS], F16, name="cache_x")
            cache_y = singles.tile([128, NJ * ROWS], F16, name="cache_y")
            cs_xy = singles.tile([128, 2 * NJ], F32, name="cs_xy")

            # ---- pass 1 ----
            def pass1(mT, mTc_sb, sqj_sb, sqi_sb, cache_sb, cs_sb, out_h, tag):
                mT_r = mT[:, :].rearrange("(k p) n -> p k n", p=128)
                for nj in range(NJ):
                    strip = strips.tile([128, NK, 128], BF16, tag="strip")
                    nc.sync.dma_start(
                        out=strip[:],
                        in_=mT_r[:, :, nj * 128 : (nj + 1) * 128],
                    )
                    ps = psum.tile([128, ROWS], F32, tag="mm")
                    for k in range(NK):
                        nc.tensor.matmul(
                            ps[:],
                            lhsT=strip[:, k, :],
                            rhs=mTc_sb[:, k, :],
                            start=(k == 0),
                            stop=(k == NK - 1),
                        )
                    u = temps.tile([128, ROWS], F32, tag="u")
                    nc.vector.tensor_scalar(
                        u[:], ps[:], -2.0, sqj_sb[:, nj : nj + 1],
                        op0=ALU.mult, op1=ALU.add,
                    )
                    v = temps.tile([128, ROWS], F32, tag="v")
                    nc.vector.tensor_add(v[:], u[:], sqi_sb[:])
                    nc.vector.tensor_scalar_max(v[:], v[:], 0.0)
                    a32 = temps.tile([128, ROWS], F32, tag="a32")
                    nc.scalar.activation(
                        a32[:], v[:], AF.Sqrt,
                        bias=eps_sb[:], scale=1.0,
                        accum_out=cs_sb[:, nj : nj + 1],
                    )
                    csl = cache_sb[:, nj * ROWS : (nj + 1) * ROWS]
                    nc.scalar.activation(csl, a32[:], AF.Copy, bias=-64.0, scale=1.0)
                    nc.scalar.dma_start(
                        out=out_h[nj * 128 : (nj + 1) * 128, :], in_=csl
                    )

            import os as _os
            STAGE = int(_os.environ.get("DCOR_STAGE", "4"))
            nc.tensor.ldweights(xTc_sb[:, 0, 0:128])
            pass1(xT, xTc_sb, sqjx_sb, sqix_sb, cache_x, cs_xy[:, 0:NJ], axh, "x")
            if STAGE >= 2:
                nc.tensor.ldweights(yTc_sb[:, 0, 0:128])
                pass1(yT, yTc_sb, sqjy_sb, sqiy_sb, cache_y, cs_xy[:, NJ : 2 * NJ], ayh, "y")

            if STAGE >= 3:
                # ---- AllReduce colsum partials ----
                cc_in = dram.tile([128, 2 * NJ], F32, name="cc_in")
                cc_out = dram.tile([128, 2 * NJ], F32, name="cc_out", addr_space="Shared")
                nc.scalar.dma_start(out=cc_in[:], in_=cs_xy[:])
                import os as _os
                if _os.environ.get("DCOR_NO_CC"):
                    nc.sync.dma_start(out=cc_out[:], in_=cc_in[:])
                else:
                    nc.gpsimd.collective_compute(
                        "AllReduce",
                        ALU.add,
                        replica_groups=[list(range(NCORES))],
                        ins=[cc_in[:]],
                        outs=[cc_out[:]],
                    )
                csf = singles.tile([128, 2 * NJ], F32, name="csf")
                nc.sync.dma_start(out=csf[:], in_=cc_out[:])

                # ---- rv_shift = -2/(n-2)*C + (S/((n-1)(n-2)) + 64) ----
                ones_row = singles.tile([1, 128], F32, name="ones_row")
                nc.vector.tensor_scalar(
                    ones_row[:], sqix_sb[0:1, 0:128], 0.0, 1.0, op0=ALU.mult, op1=ALU.add
                )
                rv_x = singles.tile([128, NJ], F32, name="rv_x")
                rv_y = singles.tile([128, NJ], F32, name="rv_y")
                for m, rv_sb in ((0, rv_x), (1, rv_y)):
                    red = temps.tile([128, 1], F32, tag="red")
                    nc.vector.tensor_reduce(
                        red[:], csf[:, m * NJ : (m + 1) * NJ], mybir.AxisListType.X, ALU.add
                    )
                    ps1 = psum1.tile([1, 1], F32, tag="ps1")
                    nc.tensor.matmul(ps1[:], lhsT=red[:], rhs=ones_sb[:], start=True, stop=True)
                    ts1 = temps.tile([1, 1], F32, tag="ts1")
                    nc.scalar.activation(
                        ts1[:], ps1[:], AF.Identity,
                        bias=c64_sb[0:1, :], scale=1.0 / ((N - 1.0) * (N - 2.0)),
                    )
                    psB = psum1.tile([128, 1], F32, tag="psB")
                    nc.tensor.matmul(psB[:], lhsT=ones_row[:], rhs=ts1[:], start=True, stop=True)
                    nc.vector.tensor_scalar(
                        rv_sb[:], csf[:, m * NJ : (m + 1) * NJ], -2.0 / (N - 2.0), psB[:],
                        op0=ALU.mult, op1=ALU.add,
                    )
                    nc.sync.dma_start(out=rvs[m], in_=rv_sb[:])

            if STAGE >= 4:
                # ---- pass 2 ----
                accs = singles.tile([128, 3 * NJ], F32, name="accs")
                for nj in range(NJ):
                    At = temps.tile([128, ROWS], F32, tag="At")
                    nc.scalar.activation(
                        At[:], cache_x[:, nj * ROWS : (nj + 1) * ROWS], AF.Identity,
                        bias=rv_x[:, nj : nj + 1], scale=1.0,
                    )
                    Bt = temps.tile([128, ROWS], F32, tag="Bt")
                    nc.scalar.activation(
                        Bt[:], cache_y[:, nj * ROWS : (nj + 1) * ROWS], AF.Identity,
                        bias=rv_y[:, nj : nj + 1], scale=1.0,
                    )
                    scrap = temps.tile([128, ROWS], F32, tag="scrap")
                    nc.vector.tensor_mul(scrap[:], At[:], Bt[:])
                    nc.vector.tensor_reduce(
                        accs[:, 0 * NJ + nj : 0 * NJ + nj + 1],
                        scrap[:], mybir.AxisListType.X, ALU.add,
                    )
                    sq_a = temps.tile([128, ROWS], F32, tag="sq_a")
                    nc.scalar.activation(
                        sq_a[:], At[:], AF.Square,
                        accum_out=accs[:, 1 * NJ + nj : 1 * NJ + nj + 1],
                    )
                    sq_b = temps.tile([128, ROWS], F32, tag="sq_b")
                    nc.scalar.activation(
                        sq_b[:], Bt[:], AF.Square,
                        accum_out=accs[:, 2 * NJ + nj : 2 * NJ + nj + 1],
                    )
                for col in range(3):
                    nc.vector.tensor_reduce(
                        acc[:, col : col + 1],
                        accs[:, col * NJ : (col + 1) * NJ],
                        mybir.AxisListType.X,
                        ALU.add,
                    )
                nc.sync.dma_start(out=pp[:, :], in_=acc[:])

    nc.compile()
    return nc


def _get_nc():
    if "nc" not in _CACHE:
        _CACHE["nc"] = _build_nc()
    return _CACHE["nc"]


def kernel(featuresX: np.ndarray, featuresY: np.ndarray) -> np.ndarray:
    X = np.asarray(featuresX, dtype=np.float32).reshape(N, D)
    Y = np.asarray(featuresY, dtype=np.float32).reshape(N, D)

    nc = _get_nc()

    sqx = np.einsum("ij,ij->i", X, X, dtype=np.float32).astype(np.float32)
    sqy = np.einsum("ij,ij->i", Y, Y, dtype=np.float32).astype(np.float32)
    xT = np.ascontiguousarray(X.T).astype(ml_dtypes.bfloat16)
    yT = np.ascontiguousarray(Y.T).astype(ml_dtypes.bfloat16)
    sqjx = np.ascontiguousarray(sqx.reshape(NJ, 128).T)
    sqjy = np.ascontiguousarray(sqy.reshape(NJ, 128).T)

    in_maps = []
    for c in range(NCORES):
        sl = slice(c * ROWS, (c + 1) * ROWS)
        in_maps.append(
            {
                "xT": xT,
                "yT": yT,
                "xTc": np.ascontiguousarray(xT[:, sl]),
                "yTc": np.ascontiguousarray(yT[:, sl]),
                "sqjx": sqjx,
                "sqjy": sqjy,
                "sqix": sqx[sl].reshape(1, ROWS),
                "sqiy": sqy[sl].reshape(1, ROWS),
            }
        )

    _CACHE["in_maps"] = in_maps
    res = run_bass_kernel_spmd(nc, in_maps, list(range(NCORES))).results

    # ---- host combine in f64 ----
    P = np.zeros(3, dtype=np.float64)
    for c in range(NCORES):
        P += res[c]["pp"][:, :3].astype(np.float64).sum(axis=0)

    rv = res[0]["rvs"]  # [2,128,NJ]; rv_flat[128*nj+p] = rv[m,p,nj]
    rvx = np.ascontiguousarray(rv[0].T).reshape(-1)
    rvy = np.ascontiguousarray(rv[1].T).reshape(-1)

    dAB = dAA = dBB = 0.0
    for c in range(NCORES):
        sl = slice(c * ROWS, (c + 1) * ROWS)
        dx16 = res[c]["axh"][sl, :].diagonal().astype(np.float32)
        dy16 = res[c]["ayh"][sl, :].diagonal().astype(np.float32)
        Adiag = (dx16 + rvx[sl]).astype(np.float32).astype(np.float64)
        Bdiag = (dy16 + rvy[sl]).astype(np.float32).astype(np.float64)
        dAB += np.sum(Adiag * Bdiag)
        dAA += np.sum(Adiag * Adiag)
        dBB += np.sum(Bdiag * Bdiag)

    denom = float(N) * (N - 3.0)
    gxy = (P[0] - dAB) / denom
    gxx = (P[1] - dAA) / denom
    gyy = (P[2] - dBB) / denom
    loss = -gxy / np.sqrt(gxx * gyy + EPS)
    return np.array(loss, dtype=np.float32)



# revision 2
# speedup vs baseline: 4.0502x; 4.0502x over previous
"""Distance-correlation loss kernel for trn2 (8 NeuronCores, SPMD).

Reference math: for F in {X, Y}: a = sqrt(relu(sq_i + sq_j - 2 F F^T) + eps),
A = a - 2*row_j + tot (row = colsum/(n-2), tot = sum/((n-1)(n-2))), zero diag;
loss = -g_xy / sqrt(g_xx * g_yy + eps), g_PQ = sum(P*Q)/(n(n-3)).

Matrix-free single-pass formulation: with a' = a - 64 and shifted colsums
ca'_j, every bracket sum expands as
  sum_{i!=j} A B = P' + O(n) correction terms from (ca', S', measured diag),
where P' = sum a'_x a'_y (and squares) — so the device only computes, per
tile, the distance tile, its shifted colsum, and three running product
partials. No second pass, no collective: the host combines per-core partials
in f64.

Device strategy per core c (tiles a^T[j_chunk=128, i=512], i = core's rows):
  fp8(e4m3) DoubleRow matmuls (4x bf16 throughput): psum = x8_strip^T (-2 x8_c)
  + an extra DoubleRow pair encoding sq_i - 2048 (two fp8 rows, exact to ~0.1);
  ACT: a = sqrt(psum + bias_j), bias_j = sq_j + 2048 + 0.5 per partition
  (+0.5 keeps the garbage diagonal positive -> no relu/mask/NaN);
  GPSIMD: a16 = a - 64 (f16) with accum_out = shifted colsum;
  DVE: 3 tensor_tensor_reduce partials (ab, aa, bb) per tile pair.
  Stream order is rotated per core (j starts at the core's own diagonal) so
  the 4 diagonal blocks sit at fixed stream positions 0..3; their a32 column
  blocks are DMA'd out and the host subtracts the exact measured diagonal.
DMA: x strips on the sync queue, y strips on the scalar queue (parallel DGE).
"""

import sys

for _p in ("/opt/trn_rl_repo",):
    if _p not in sys.path:
        sys.path.insert(0, _p)

import numpy as np
import ml_dtypes

import concourse.bass as bass
from concourse import bacc
import concourse.mybir as mybir
import concourse.tile as tile
from concourse.bass_utils import run_bass_kernel_spmd

N = 4096
D = 2048
NCORES = 8
ROWS = N // NCORES          # 512 free-dim rows per core
NJ = N // 128               # 32 j-chunks (partition dim of transposed tiles)
NKP = D // 256              # 8 DoubleRow contraction pairs
K64 = 64.0
EB = 0.5                    # bias pad keeping the junk diagonal positive
EPS = 1e-18
F32 = mybir.dt.float32
F16 = mybir.dt.float16
F8 = mybir.dt.float8e4
AF = mybir.ActivationFunctionType
ALU = mybir.AluOpType
DR = mybir.MatmulPerfMode.DoubleRow
f8 = ml_dtypes.float8_e4m3

_CACHE = {}


def _build_nc():
    nc = bacc.Bacc(None, num_devices=NCORES, target_bir_lowering=False)

    # ---- inputs (per core) ----
    # streamed j-side, columns rotated by 512*c so stream pos 0 = own diagonal
    xs8 = nc.declare_dram_parameter("xs8", [D, N], F8, isOutput=False)
    ys8 = nc.declare_dram_parameter("ys8", [D, N], F8, isOutput=False)
    # resident moving side: (-2 x8)^T restricted to the core's 512 rows
    xm8 = nc.declare_dram_parameter("xm8", [D, ROWS], F8, isOutput=False)
    ym8 = nc.declare_dram_parameter("ym8", [D, ROWS], F8, isOutput=False)
    # fp8 encoding of sq_i - 2048 (two rows: u/16 and residual/2)
    uabx = nc.declare_dram_parameter("uabx", [1, 2 * ROWS], F8, isOutput=False)
    uaby = nc.declare_dram_parameter("uaby", [1, 2 * ROWS], F8, isOutput=False)
    stat = nc.declare_dram_parameter("stat", [1, 256], F8, isOutput=False)
    # per-partition sqrt bias by stream position: sq_j + 2048 + EB
    sqbx = nc.declare_dram_parameter("sqbx", [128, NJ], F32, isOutput=False)
    sqby = nc.declare_dram_parameter("sqby", [128, NJ], F32, isOutput=False)

    # ---- outputs ----
    csx = nc.declare_dram_parameter("csx", [128, NJ], F32, isOutput=True)
    csy = nc.declare_dram_parameter("csy", [128, NJ], F32, isOutput=True)
    accs = nc.declare_dram_parameter("accs", [128, 3 * NJ], F32, isOutput=True)
    adx = nc.declare_dram_parameter("adx", [128, 512], F32, isOutput=True)
    ady = nc.declare_dram_parameter("ady", [128, 512], F32, isOutput=True)

    with tile.TileContext(nc) as tc:
        import contextlib

        with contextlib.ExitStack() as ctx:
            singles = ctx.enter_context(tc.tile_pool(name="singles", bufs=1))
            xstrips = ctx.enter_context(tc.tile_pool(name="xstrips", bufs=2))
            ystrips = ctx.enter_context(tc.tile_pool(name="ystrips", bufs=2))
            psum = ctx.enter_context(tc.tile_pool(name="psum", bufs=4, space="PSUM"))
            t32 = ctx.enter_context(tc.tile_pool(name="t32", bufs=4))
            t16 = ctx.enter_context(tc.tile_pool(name="t16", bufs=4))
            scrap = ctx.enter_context(tc.tile_pool(name="scrap", bufs=3))

            # ---- residents (gpsimd DMA queue) ----
            xm_sb = singles.tile([128, D // 128, ROWS], F8, name="xm_sb")
            nc.gpsimd.dma_start(
                out=xm_sb[:], in_=xm8[:, :].rearrange("(k p) i -> p k i", p=128)
            )
            ym_sb = singles.tile([128, D // 128, ROWS], F8, name="ym_sb")
            nc.gpsimd.dma_start(
                out=ym_sb[:], in_=ym8[:, :].rearrange("(k p) i -> p k i", p=128)
            )
            sqbx_sb = singles.tile([128, NJ], F32, name="sqbx_sb")
            nc.gpsimd.dma_start(out=sqbx_sb[:], in_=sqbx[:, :])
            sqby_sb = singles.tile([128, NJ], F32, name="sqby_sb")
            nc.gpsimd.dma_start(out=sqby_sb[:], in_=sqby[:, :])
            uabx_sb = singles.tile([1, 2, ROWS], F8, name="uabx_sb")
            nc.gpsimd.dma_start(out=uabx_sb[:], in_=uabx[:, :])
            uaby_sb = singles.tile([1, 2, ROWS], F8, name="uaby_sb")
            nc.gpsimd.dma_start(out=uaby_sb[:], in_=uaby[:, :])
            stat_sb = singles.tile([1, 2, 128], F8, name="stat_sb")
            nc.gpsimd.dma_start(out=stat_sb[:], in_=stat[:, :])

            csx_sb = singles.tile([128, NJ], F32, name="csx_sb")
            csy_sb = singles.tile([128, NJ], F32, name="csy_sb")
            accs_sb = singles.tile([128, 3 * NJ], F32, name="accs_sb")

            sides = (
                (xm_sb, uabx_sb, sqbx_sb, csx_sb, adx),
                (ym_sb, uaby_sb, sqby_sb, csy_sb, ady),
            )

            for s in range(N // 512):
                w0 = 512 * s
                xst = xstrips.tile([128, D // 128, 512], F8, tag="xs")
                nc.sync.dma_start(
                    out=xst[:],
                    in_=xs8[:, w0 : w0 + 512].rearrange("(k p) n -> p k n", p=128),
                )
                yst = ystrips.tile([128, D // 128, 512], F8, tag="ys")
                nc.scalar.dma_start(
                    out=yst[:],
                    in_=ys8[:, w0 : w0 + 512].rearrange("(k p) n -> p k n", p=128),
                )
                for t in range(4):
                    pos = 4 * s + t
                    a16s = []
                    for m, (m_sb, uab_sb, sqb_sb, cs_sb, ad) in enumerate(sides):
                        strip = xst if m == 0 else yst
                        ps = psum.tile([128, 512], F32, tag="mm")
                        for kp in range(NKP):
                            nc.tensor.matmul(
                                ps[:],
                                lhsT=strip[:, 2 * kp : 2 * kp + 2, 128 * t : 128 * t + 128],
                                rhs=m_sb[:, 2 * kp : 2 * kp + 2, :],
                                start=(kp == 0),
                                stop=False,
                                perf_mode=DR,
                            )
                        nc.tensor.matmul(
                            ps[:], lhsT=stat_sb[:], rhs=uab_sb[:],
                            start=False, stop=True, perf_mode=DR,
                        )
                        a32 = t32.tile([128, 512], F32, tag="a32")
                        nc.scalar.activation(
                            a32[:], ps[:], AF.Sqrt,
                            bias=sqb_sb[:, pos : pos + 1], scale=1.0,
                        )
                        a16 = t16.tile([128, 512], F16, tag="a16")
                        nc.gpsimd.tensor_scalar(
                            a16[:], a32[:], -K64, None,
                            op0=ALU.add, op1=ALU.add,
                            accum_out=cs_sb[:, pos : pos + 1],
                        )
                        a16s.append(a16)
                        if s == 0:
                            nc.sync.dma_start(
                                out=ad[:, 128 * t : 128 * t + 128],
                                in_=a32[:, 128 * t : 128 * t + 128],
                            )
                    a16x, a16y = a16s
                    for col, (i0, i1) in enumerate(
                        ((a16x, a16y), (a16x, a16x), (a16y, a16y))
                    ):
                        scr = scrap.tile([128, 512], F16, tag="scr")
                        nc.vector.tensor_tensor_reduce(
                            out=scr[:], in0=i0[:], in1=i1[:],
                            scale=1.0, scalar=0.0,
                            op0=ALU.mult, op1=ALU.add,
                            accum_out=accs_sb[:, col * NJ + pos : col * NJ + pos + 1],
                        )

            nc.sync.dma_start(out=csx[:, :], in_=csx_sb[:])
            nc.sync.dma_start(out=csy[:, :], in_=csy_sb[:])
            nc.sync.dma_start(out=accs[:, :], in_=accs_sb[:])

    nc.compile()
    return nc


def _get_nc():
    if "nc" not in _CACHE:
        _CACHE["nc"] = _build_nc()
    return _CACHE["nc"]


def _prep_side(F):
    """Host prep for one feature matrix: fp8 cast + derived arrays."""
    x8 = np.asarray(F, dtype=np.float32).reshape(N, D).astype(f8).astype(np.float32)
    xsT = np.ascontiguousarray(x8.T).astype(f8)                 # [D, N]
    xmT = np.ascontiguousarray((-2.0 * x8).T).astype(f8)        # [D, N] (sliced per core)
    sq = np.einsum("ij,ij->i", x8.astype(np.float64), x8.astype(np.float64))
    u = sq - 2048.0
    uA = (u / 16.0).astype(f8)
    uB = ((u - uA.astype(np.float64) * 16.0) / 2.0).astype(f8)
    sqb = (sq + 2048.0 + EB).astype(np.float32)                 # [N]
    return xsT, xmT, np.asarray(uA), np.asarray(uB), sqb


def _make_in_maps(featuresX, featuresY):
    xsT, xmT, uAx, uBx, sqbx = _prep_side(featuresX)
    ysT, ymT, uAy, uBy, sqby = _prep_side(featuresY)
    stat_np = np.concatenate(
        [np.full(128, 16.0, np.float32), np.full(128, 2.0, np.float32)]
    ).astype(f8).reshape(1, 256)

    in_maps = []
    for c in range(NCORES):
        sl = slice(c * ROWS, (c + 1) * ROWS)
        rot = np.r_[4 * c : NJ, 0 : 4 * c]                      # stream pos -> global chunk
        in_maps.append(
            {
                "xs8": np.ascontiguousarray(np.roll(xsT, -512 * c, axis=1)),
                "ys8": np.ascontiguousarray(np.roll(ysT, -512 * c, axis=1)),
                "xm8": np.ascontiguousarray(xmT[:, sl]),
                "ym8": np.ascontiguousarray(ymT[:, sl]),
                "uabx": np.concatenate([uAx[sl], uBx[sl]]).reshape(1, 2 * ROWS),
                "uaby": np.concatenate([uAy[sl], uBy[sl]]).reshape(1, 2 * ROWS),
                "stat": stat_np,
                "sqbx": np.ascontiguousarray(sqbx.reshape(NJ, 128).T[:, rot]),
                "sqby": np.ascontiguousarray(sqby.reshape(NJ, 128).T[:, rot]),
            }
        )
    return in_maps


def _combine(res):
    """f64 host combine of per-core partials -> loss."""
    cspx = np.zeros(N, np.float64)
    cspy = np.zeros(N, np.float64)
    P = np.zeros(3, np.float64)                                  # ab, aa, bb
    adiag_x = np.zeros(N, np.float64)
    adiag_y = np.zeros(N, np.float64)
    for c in range(NCORES):
        r = res[c]
        for pos in range(NJ):
            gj = (4 * c + pos) % NJ
            cspx[128 * gj : 128 * gj + 128] += r["csx"][:, pos].astype(np.float64)
            cspy[128 * gj : 128 * gj + 128] += r["csy"][:, pos].astype(np.float64)
        P += r["accs"].astype(np.float64).reshape(128, 3, NJ).sum(axis=(0, 2))
        for t in range(4):
            blk_x = r["adx"][:, 128 * t : 128 * t + 128]
            blk_y = r["ady"][:, 128 * t : 128 * t + 128]
            i0 = 512 * c + 128 * t
            adiag_x[i0 : i0 + 128] = np.diagonal(blk_x).astype(np.float64)
            adiag_y[i0 : i0 + 128] = np.diagonal(blk_y).astype(np.float64)

    def bracket(Pv, c1p, c2p, d1, d2_):
        n = float(N)
        r1 = c1p / (n - 2)
        r2 = c2p / (n - 2)
        t1 = c1p.sum() / ((n - 1) * (n - 2)) - K64 / (n - 1)
        t2 = c2p.sum() / ((n - 1) * (n - 2)) - K64 / (n - 1)
        sv = Pv
        sv += -2.0 * (r2 @ c1p) + t2 * c1p.sum()
        sv += -2.0 * (r1 @ c2p) + t1 * c2p.sum()
        sv += 4.0 * n * (r1 @ r2)
        sv += -2.0 * n * t2 * r1.sum() - 2.0 * n * t1 * r2.sum()
        sv += n * n * t1 * t2
        A_ii = (d1 - K64) - 2.0 * r1 + t1
        B_ii = (d2_ - K64) - 2.0 * r2 + t2
        sv -= (A_ii * B_ii).sum()
        return sv / (n * (n - 3.0))

    gxy = bracket(P[0], cspx, cspy, adiag_x, adiag_y)
    gxx = bracket(P[1], cspx, cspx, adiag_x, adiag_x)
    gyy = bracket(P[2], cspy, cspy, adiag_y, adiag_y)
    loss = -gxy / np.sqrt(gxx * gyy + EPS)
    return np.array(loss, dtype=np.float32)


def kernel(featuresX: np.ndarray, featuresY: np.ndarray) -> np.ndarray:
    nc = _get_nc()
    in_maps = _make_in_maps(featuresX, featuresY)
    _CACHE["in_maps"] = in_maps
    res = run_bass_kernel_spmd(nc, in_maps, list(range(NCORES))).results
    return _combine(res)


# revision 4
# speedup vs baseline: 5.6661x; 1.3990x over previous
"""Distance-correlation loss kernel for trn2 (8 NeuronCores, SPMD).

Reference math: for F in {X, Y}: a = sqrt(relu(sq_i + sq_j - 2 F F^T) + eps),
A = a - 2*row_j + tot (row = colsum/(n-2), tot = sum/((n-1)(n-2))), zero diag;
loss = -g_xy / sqrt(g_xx * g_yy + eps), g_PQ = sum(P*Q)/(n(n-3)).

Matrix-free single-pass formulation: with a' = a - 64, every bracket sum
expands as P' (= sum a'_x a'_y and squares) plus O(n) corrections from the
shifted colsums and the measured diagonal — so the device computes, per tile,
only the distance tile, its shifted colsum, and three running product
partials. No second pass, no collective; host combines per-core f64 partials.

Symmetric schedule (a is symmetric -> compute ~half the matrix): 16 virtual
half-blocks of 256 rows, 2 per core. Core c streams 20 rotated j-chunks
(global chunk (4c+pos)%32); per stream position the tile is
  pos 0,1:   [128,256] left  (v0 self, weight 1)
  pos 2,3:   [128,512] left w2 + right v1-self w1
  pos 4..15: [128,512] both halves w2
  pos 16,17: [128,512] left w1 (d=8 pair, both orientations computed), right w2
  pos 18,19: [128,256] right w1 (odd d=8 pair)
Weight-2 halves get transposed-side column sums via ones^T @ a16 matmuls
accumulated in persistent PSUM banks (emitted with a 2-position lag so the
PE never waits on the ACT/GPSIMD chain). Weights are baked into the DVE
tensor_tensor_reduce `scale`.

Per tile: fp8(e4m3) DoubleRow matmuls (psum = x8_strip^T (-2 x8_core)) plus
one DoubleRow pair encoding sq_i - 2048; ACT sqrt with per-partition bias
sq_j + 2048 + 0.5 (+0.5 keeps the junk diagonal positive -> no relu/NaN);
GPSIMD tensor_scalar shift a-64 -> f16 with accum_out = shifted colsum; DVE
TTR partials. Diagonal blocks sit at stream positions 0..3 (rotation starts
at the core's own rows); their a32 column blocks are DMA'd out and the host
subtracts the exact measured diagonal. x-strips ride the SP DMA queue,
y-strips the GPSIMD queue.
"""

import sys

for _p in ("/opt/trn_rl_repo",):
    if _p not in sys.path:
        sys.path.insert(0, _p)

import numpy as np
import ml_dtypes

import concourse.bass as bass
from concourse import bacc
import concourse.mybir as mybir
import concourse.tile as tile
from concourse.bass_utils import run_bass_kernel_spmd

N = 4096
D = 2048
NCORES = 8
ROWS = N // NCORES          # 512 resident rows per core
NJ = N // 128               # 32 global j-chunks
NPOS = 20                   # streamed chunk positions per core
NSTRIP = NPOS // 4          # 5 strips of 512 columns
NKP = D // 256              # 8 DoubleRow contraction pairs
K64 = 64.0
EB = 0.5
EPS = 1e-18
F32 = mybir.dt.float32
F16 = mybir.dt.float16
F8 = mybir.dt.float8e4
AF = mybir.ActivationFunctionType
ALU = mybir.AluOpType
DR = mybir.MatmulPerfMode.DoubleRow
f8 = ml_dtypes.float8_e4m3

_CACHE = {}


def _pos_tile(pos):
    """(tile_col_start, tile_width) within the core's 512 resident columns."""
    if pos < 2:
        return 0, 256
    if pos >= 18:
        return 256, 256
    return 0, 512


def _pos_ttrs(pos):
    """List of (col_start, width, weight) product segments for this position."""
    if pos < 2:
        return [(0, 256, 1.0)]
    if pos < 4:
        return [(0, 256, 2.0), (256, 256, 1.0)]
    if pos < 16:
        return [(0, 512, 2.0)]
    if pos < 18:
        return [(0, 256, 1.0), (256, 256, 2.0)]
    return [(256, 256, 1.0)]


def _pos_mirrors(pos):
    """Mirror groups fed at this position: list of (group, col_start)."""
    out = []
    if 2 <= pos <= 15:
        out.append((0, 0))
    if 4 <= pos <= 17:
        out.append((1, 256))
    return out


NACC = sum(len(_pos_ttrs(p)) for p in range(NPOS))   # accum columns per product


def _build_nc():
    nc = bacc.Bacc(None, num_devices=NCORES, target_bir_lowering=False)

    # ---- inputs ----
    xs8 = nc.declare_dram_parameter("xs8", [D, NPOS * 128], F8, isOutput=False)
    ys8 = nc.declare_dram_parameter("ys8", [D, NPOS * 128], F8, isOutput=False)
    xm8 = nc.declare_dram_parameter("xm8", [D, ROWS], F8, isOutput=False)
    ym8 = nc.declare_dram_parameter("ym8", [D, ROWS], F8, isOutput=False)
    uabx = nc.declare_dram_parameter("uabx", [1, 2 * ROWS], F8, isOutput=False)
    uaby = nc.declare_dram_parameter("uaby", [1, 2 * ROWS], F8, isOutput=False)
    stat = nc.declare_dram_parameter("stat", [1, 256], F8, isOutput=False)
    onesf = nc.declare_dram_parameter("onesf", [128, 1], F16, isOutput=False)
    sqbx = nc.declare_dram_parameter("sqbx", [128, NPOS], F32, isOutput=False)
    sqby = nc.declare_dram_parameter("sqby", [128, NPOS], F32, isOutput=False)

    # ---- outputs ----
    csx = nc.declare_dram_parameter("csx", [128, NPOS], F32, isOutput=True)
    csy = nc.declare_dram_parameter("csy", [128, NPOS], F32, isOutput=True)
    accs = nc.declare_dram_parameter("accs", [128, 3 * NACC], F32, isOutput=True)
    adx = nc.declare_dram_parameter("adx", [128, 512], F32, isOutput=True)
    ady = nc.declare_dram_parameter("ady", [128, 512], F32, isOutput=True)
    mirs = nc.declare_dram_parameter("mirs", [1, 1024], F32, isOutput=True)

    with tile.TileContext(nc) as tc:
        import contextlib

        with contextlib.ExitStack() as ctx:
            singles = ctx.enter_context(tc.tile_pool(name="singles", bufs=1))
            xstrips = ctx.enter_context(tc.tile_pool(name="xstrips", bufs=2))
            ystrips = ctx.enter_context(tc.tile_pool(name="ystrips", bufs=2))
            psum = ctx.enter_context(tc.tile_pool(name="psum", bufs=4, space="PSUM"))
            mpsum = ctx.enter_context(tc.tile_pool(name="mpsum", bufs=1, space="PSUM"))
            t32 = ctx.enter_context(tc.tile_pool(name="t32", bufs=4))
            t16 = ctx.enter_context(tc.tile_pool(name="t16", bufs=8))
            scrap = ctx.enter_context(tc.tile_pool(name="scrap", bufs=3))

            # ---- residents: x-side on SP queue, y-side on gpsimd queue ----
            xm_sb = singles.tile([128, D // 128, ROWS], F8, name="xm_sb")
            nc.sync.dma_start(
                out=xm_sb[:], in_=xm8[:, :].rearrange("(k p) i -> p k i", p=128)
            )
            ym_sb = singles.tile([128, D // 128, ROWS], F8, name="ym_sb")
            nc.gpsimd.dma_start(
                out=ym_sb[:], in_=ym8[:, :].rearrange("(k p) i -> p k i", p=128)
            )
            sqbx_sb = singles.tile([128, NPOS], F32, name="sqbx_sb")
            nc.sync.dma_start(out=sqbx_sb[:], in_=sqbx[:, :])
            sqby_sb = singles.tile([128, NPOS], F32, name="sqby_sb")
            nc.gpsimd.dma_start(out=sqby_sb[:], in_=sqby[:, :])
            uabx_sb = singles.tile([1, 2, ROWS], F8, name="uabx_sb")
            nc.sync.dma_start(out=uabx_sb[:], in_=uabx[:, :])
            uaby_sb = singles.tile([1, 2, ROWS], F8, name="uaby_sb")
            nc.gpsimd.dma_start(out=uaby_sb[:], in_=uaby[:, :])
            stat_sb = singles.tile([1, 2, 128], F8, name="stat_sb")
            nc.sync.dma_start(out=stat_sb[:], in_=stat[:, :])
            ones_sb = singles.tile([128, 1], F16, name="ones_sb")
            nc.sync.dma_start(out=ones_sb[:], in_=onesf[:, :])

            csx_sb = singles.tile([128, NPOS], F32, name="csx_sb")
            csy_sb = singles.tile([128, NPOS], F32, name="csy_sb")
            accs_sb = singles.tile([128, 3 * NACC], F32, name="accs_sb")
            mir_sb = singles.tile([1, 1024], F32, name="mir_sb")

            # persistent mirror psum banks: [m][group]
            mir_ps = [
                [mpsum.tile([1, 512], F32, name=f"mir{m}{g}") for g in range(2)]
                for m in range(2)
            ]
            mir_started = [[False, False], [False, False]]
            mir_count = [[0, 0], [0, 0]]
            MIR_TOTAL = [14, 14]   # group 0: pos 2..15; group 1: pos 4..17
            pending_mirrors = []   # (m, group, a16_tile, col_start)

            sides = (
                (xm_sb, uabx_sb, sqbx_sb, csx_sb, adx),
                (ym_sb, uaby_sb, sqby_sb, csy_sb, ady),
            )

            acc_col = 0
            for s in range(NSTRIP):
                w0 = 512 * s
                xst = xstrips.tile([128, D // 128, 512], F8, tag="xs")
                nc.sync.dma_start(
                    out=xst[:],
                    in_=xs8[:, w0 : w0 + 512].rearrange("(k p) n -> p k n", p=128),
                )
                yst = ystrips.tile([128, D // 128, 512], F8, tag="ys")
                nc.gpsimd.dma_start(
                    out=yst[:],
                    in_=ys8[:, w0 : w0 + 512].rearrange("(k p) n -> p k n", p=128),
                )
                for t in range(4):
                    pos = 4 * s + t
                    c0, cw = _pos_tile(pos)
                    a16s = []
                    for m, (m_sb, uab_sb, sqb_sb, cs_sb, ad) in enumerate(sides):
                        strip = xst if m == 0 else yst
                        ps = psum.tile([128, cw], F32, tag="mm")
                        for kp in range(NKP):
                            nc.tensor.matmul(
                                ps[:],
                                lhsT=strip[:, 2 * kp : 2 * kp + 2, 128 * t : 128 * t + 128],
                                rhs=m_sb[:, 2 * kp : 2 * kp + 2, c0 : c0 + cw],
                                start=(kp == 0),
                                stop=False,
                                perf_mode=DR,
                            )
                        nc.tensor.matmul(
                            ps[:], lhsT=stat_sb[:], rhs=uab_sb[:, :, c0 : c0 + cw],
                            start=False, stop=True, perf_mode=DR,
                        )
                        a32 = t32.tile([128, cw], F32, tag="a32")
                        nc.scalar.activation(
                            a32[:], ps[:], AF.Sqrt,
                            bias=sqb_sb[:, pos : pos + 1], scale=1.0,
                        )
                        a16 = t16.tile([128, cw], F16, tag="a16")
                        nc.gpsimd.tensor_scalar(
                            a16[:], a32[:], -K64, None,
                            op0=ALU.add, op1=ALU.add,
                            accum_out=cs_sb[:, pos : pos + 1],
                        )
                        a16s.append(a16)
                        if pos < 4:
                            nc.sync.dma_start(
                                out=ad[:, 128 * pos : 128 * pos + 128],
                                in_=a32[:, 128 * pos - c0 : 128 * pos - c0 + 128],
                            )
                        for g, gcol in _pos_mirrors(pos):
                            pending_mirrors.append((m, g, a16, gcol - c0))

                    a16x, a16y = a16s
                    for seg0, segw, segwt in _pos_ttrs(pos):
                        for k, (i0t, i1t) in enumerate(
                            ((a16x, a16y), (a16x, a16x), (a16y, a16y))
                        ):
                            scr = scrap.tile([128, segw], F16, tag="scr")
                            nc.vector.tensor_tensor_reduce(
                                out=scr[:],
                                in0=i0t[:, seg0 - c0 : seg0 - c0 + segw],
                                in1=i1t[:, seg0 - c0 : seg0 - c0 + segw],
                                scale=segwt, scalar=0.0,
                                op0=ALU.mult, op1=ALU.add,
                                accum_out=accs_sb[:, k * NACC + acc_col : k * NACC + acc_col + 1],
                            )
                        acc_col += 1

                    # flush mirror matmuls with a 2-position lag so PE never
                    # waits on the ACT/GPSIMD chain
                    while len(pending_mirrors) > 4:
                        m, g, a16t, rel = pending_mirrors.pop(0)
                        mir_count[m][g] += 1
                        nc.tensor.matmul(
                            mir_ps[m][g][:, 0:256],
                            lhsT=ones_sb[:],
                            rhs=a16t[:, rel : rel + 256],
                            start=not mir_started[m][g],
                            stop=(mir_count[m][g] == MIR_TOTAL[g]),
                        )
                        mir_started[m][g] = True

            for m, g, a16t, rel in pending_mirrors:
                mir_count[m][g] += 1
                nc.tensor.matmul(
                    mir_ps[m][g][:, 0:256],
                    lhsT=ones_sb[:],
                    rhs=a16t[:, rel : rel + 256],
                    start=not mir_started[m][g],
                    stop=(mir_count[m][g] == MIR_TOTAL[g]),
                )
                mir_started[m][g] = True

            # evict mirror psums: layout [x_g0 | x_g1 | y_g0 | y_g1]
            for m in range(2):
                for g in range(2):
                    o = 256 * (2 * m + g)
                    nc.scalar.activation(
                        mir_sb[:, o : o + 256], mir_ps[m][g][:, 0:256],
                        AF.Copy, bias=0.0, scale=1.0,
                    )

            nc.sync.dma_start(out=csx[:, :], in_=csx_sb[:])
            nc.sync.dma_start(out=csy[:, :], in_=csy_sb[:])
            nc.sync.dma_start(out=accs[:, :], in_=accs_sb[:])
            nc.sync.dma_start(out=mirs[:, :], in_=mir_sb[:])

    nc.compile()
    return nc


def _get_nc():
    if "nc" not in _CACHE:
        _CACHE["nc"] = _build_nc()
    return _CACHE["nc"]


def _prep_side(F):
    x8 = np.asarray(F, dtype=np.float32).reshape(N, D).astype(f8).astype(np.float32)
    xsT = np.ascontiguousarray(x8.T).astype(f8)                 # [D, N]
    xmT = np.ascontiguousarray((-2.0 * x8).T).astype(f8)        # [D, N]
    sq = np.einsum("ij,ij->i", x8.astype(np.float64), x8.astype(np.float64))
    u = sq - 2048.0
    uA = (u / 16.0).astype(f8)
    uB = ((u - uA.astype(np.float64) * 16.0) / 2.0).astype(f8)
    sqb = (sq + 2048.0 + EB).astype(np.float32)
    return xsT, xmT, np.asarray(uA), np.asarray(uB), sqb


def _rot_window(arr, c):
    """Columns (512c .. 512c + NPOS*128) mod N of [D, N] array."""
    start = 512 * c
    end = start + NPOS * 128
    if end <= N:
        return np.ascontiguousarray(arr[:, start:end])
    return np.ascontiguousarray(
        np.concatenate([arr[:, start:], arr[:, : end - N]], axis=1)
    )


def _make_in_maps(featuresX, featuresY):
    xsT, xmT, uAx, uBx, sqbx = _prep_side(featuresX)
    ysT, ymT, uAy, uBy, sqby = _prep_side(featuresY)
    stat_np = np.concatenate(
        [np.full(128, 16.0, np.float32), np.full(128, 2.0, np.float32)]
    ).astype(f8).reshape(1, 256)
    ones_np = np.ones((128, 1), np.float16)

    in_maps = []
    for c in range(NCORES):
        sl = slice(c * ROWS, (c + 1) * ROWS)
        rot = [(4 * c + pos) % NJ for pos in range(NPOS)]
        sqbx_c = np.stack([sqbx[128 * g : 128 * g + 128] for g in rot], axis=1)
        sqby_c = np.stack([sqby[128 * g : 128 * g + 128] for g in rot], axis=1)
        in_maps.append(
            {
                "xs8": _rot_window(xsT, c),
                "ys8": _rot_window(ysT, c),
                "xm8": np.ascontiguousarray(xmT[:, sl]),
                "ym8": np.ascontiguousarray(ymT[:, sl]),
                "uabx": np.concatenate([uAx[sl], uBx[sl]]).reshape(1, 2 * ROWS),
                "uaby": np.concatenate([uAy[sl], uBy[sl]]).reshape(1, 2 * ROWS),
                "stat": stat_np,
                "onesf": ones_np,
                "sqbx": np.ascontiguousarray(sqbx_c),
                "sqby": np.ascontiguousarray(sqby_c),
            }
        )
    return in_maps


def _combine(res):
    cspx = np.zeros(N, np.float64)
    cspy = np.zeros(N, np.float64)
    P = np.zeros(3, np.float64)
    adiag_x = np.zeros(N, np.float64)
    adiag_y = np.zeros(N, np.float64)
    for c in range(NCORES):
        r = res[c]
        for pos in range(NPOS):
            gj = (4 * c + pos) % NJ
            cspx[128 * gj : 128 * gj + 128] += r["csx"][:, pos].astype(np.float64)
            cspy[128 * gj : 128 * gj + 128] += r["csy"][:, pos].astype(np.float64)
        P += r["accs"].astype(np.float64).reshape(128, 3, NACC).sum(axis=(0, 2))
        mir = r["mirs"].astype(np.float64).reshape(4, 256)
        i0 = 512 * c
        cspx[i0 : i0 + 256] += mir[0]
        cspx[i0 + 256 : i0 + 512] += mir[1]
        cspy[i0 : i0 + 256] += mir[2]
        cspy[i0 + 256 : i0 + 512] += mir[3]
        for t in range(4):
            blk_x = r["adx"][:, 128 * t : 128 * t + 128]
            blk_y = r["ady"][:, 128 * t : 128 * t + 128]
            adiag_x[i0 + 128 * t : i0 + 128 * t + 128] = np.diagonal(blk_x).astype(np.float64)
            adiag_y[i0 + 128 * t : i0 + 128 * t + 128] = np.diagonal(blk_y).astype(np.float64)

    def bracket(Pv, c1p, c2p, d1, d2_):
        n = float(N)
        r1 = c1p / (n - 2)
        r2 = c2p / (n - 2)
        t1 = c1p.sum() / ((n - 1) * (n - 2)) - K64 / (n - 1)
        t2 = c2p.sum() / ((n - 1) * (n - 2)) - K64 / (n - 1)
        sv = Pv
        sv += -2.0 * (r2 @ c1p) + t2 * c1p.sum()
        sv += -2.0 * (r1 @ c2p) + t1 * c2p.sum()
        sv += 4.0 * n * (r1 @ r2)
        sv += -2.0 * n * t2 * r1.sum() - 2.0 * n * t1 * r2.sum()
        sv += n * n * t1 * t2
        A_ii = (d1 - K64) - 2.0 * r1 + t1
        B_ii = (d2_ - K64) - 2.0 * r2 + t2
        sv -= (A_ii * B_ii).sum()
        return sv / (n * (n - 3.0))

    gxy = bracket(P[0], cspx, cspy, adiag_x, adiag_y)
    gxx = bracket(P[1], cspx, cspx, adiag_x, adiag_x)
    gyy = bracket(P[2], cspy, cspy, adiag_y, adiag_y)
    loss = -gxy / np.sqrt(gxx * gyy + EPS)
    return np.array(loss, dtype=np.float32)


def kernel(featuresX: np.ndarray, featuresY: np.ndarray) -> np.ndarray:
    nc = _get_nc()
    in_maps = _make_in_maps(featuresX, featuresY)
    _CACHE["in_maps"] = in_maps
    res = run_bass_kernel_spmd(nc, in_maps, list(range(NCORES))).results
    return _combine(res)


# revision 8
# speedup vs baseline: 5.8971x; 1.0408x over previous
"""Distance-correlation loss kernel for trn2 (8 NeuronCores, SPMD).

Reference math: for F in {X, Y}: a = sqrt(relu(sq_i + sq_j - 2 F F^T) + eps),
A = a - 2*row_j + tot (row = colsum/(n-2), tot = sum/((n-1)(n-2))), zero diag;
loss = -g_xy / sqrt(g_xx * g_yy + eps), g_PQ = sum(P*Q)/(n(n-3)).

Matrix-free single-pass formulation: with a' = a - 64, every bracket sum
expands as P' (= sum a'_x a'_y and squares) plus O(n) corrections from the
shifted colsums and the measured diagonal — the device computes, per tile,
only the distance tile, its shifted colsum, and three product partials. No
second pass, no collective; the host combines per-core partials in f64.

Symmetric schedule (a is symmetric -> compute ~half): 16 virtual half-blocks
of 256 rows, 2 per core. Core c streams 20 rotated j-chunks (global chunk
(4c+pos)%32); per position the tile is
  pos 0,1:   [128,256] left  (v0 self, weight 1)
  pos 2,3:   [128,512] left w2 + right v1-self w1
  pos 4..15: [128,512] both halves w2
  pos 16,17: [128,512] left w1 (d=8 pair, both orientations), right w2
  pos 18,19: [128,256] right w1 (odd d=8 pair)
Weight-2 halves get transposed-side column sums via ones^T @ a16 matmuls
accumulated in one shared PSUM bank (groups at partitions 0/32/64/96),
emitted with a lag so PE never waits on the ACT/GPSIMD chain. Weights are
baked into the DVE tensor_tensor_reduce `scale`.

Per tile: fp8(e4m3) DoubleRow matmuls (psum = x8_strip^T (-2 x8_core)) plus
one DoubleRow pair encoding sq_i - 2048; ACT sqrt with per-partition bias
sq_j + 2048 + 0.5 (+0.5 keeps the junk diagonal positive -> no relu/NaN);
GPSIMD tensor_scalar shift a-64 -> f16 with accum_out = shifted colsum; DVE
TTR partials. Diagonal blocks sit at stream positions 0..3; their a32 column
blocks are DMA'd out and the host subtracts the exact measured diagonal.

Pipelining: host pre-arranges every fp8 array in exact SBUF layout (fully
contiguous per partition -> no sub-512B DMA penalty); strips are half-width
(1.6us each) for fine overlap; x-strips ride SP, residents the scalar queue,
small tables + y-strips gpsimd; the x stream runs SKEW positions ahead of y
so the PE starts as soon as the first x strip lands.
"""

import sys

for _p in ("/opt/trn_rl_repo",):
    if _p not in sys.path:
        sys.path.insert(0, _p)

import numpy as np
import ml_dtypes

import concourse.bass as bass
from concourse import bacc
import concourse.mybir as mybir
import concourse.tile as tile
from concourse.bass_utils import run_bass_kernel_spmd

N = 4096
D = 2048
NCORES = 8
ROWS = N // NCORES          # 512 resident rows per core
NJ = N // 128               # 32 global j-chunks
NPOS = 20                   # streamed chunk positions per core
NKP = D // 256              # 8 DoubleRow contraction pairs
SKEW = 3                    # x stream runs this many positions ahead of y
K64 = 64.0
EB = 0.5
EPS = 1e-18
F32 = mybir.dt.float32
F16 = mybir.dt.float16
F8 = mybir.dt.float8e4
AF = mybir.ActivationFunctionType
ALU = mybir.AluOpType
DR = mybir.MatmulPerfMode.DoubleRow
f8 = ml_dtypes.float8_e4m3

_CACHE = {}


def _pos_tile(pos):
    """(tile_col_start, tile_width) within the core's 512 resident columns."""
    if pos < 2:
        return 0, 256
    if pos >= 18:
        return 256, 256
    return 0, 512


def _pos_ttrs(pos):
    """(col_start, width, weight) product segments for this position."""
    if pos < 2:
        return [(0, 256, 1.0)]
    if pos < 4:
        return [(0, 256, 2.0), (256, 256, 1.0)]
    if pos < 16:
        return [(0, 512, 2.0)]
    if pos < 18:
        return [(0, 256, 1.0), (256, 256, 2.0)]
    return [(256, 256, 1.0)]


def _pos_mirrors(pos):
    """Mirror groups fed at this position: list of (group, col_start)."""
    out = []
    if 2 <= pos <= 15:
        out.append((0, 0))
    if 4 <= pos <= 17:
        out.append((1, 256))
    return out


NACC = sum(len(_pos_ttrs(p)) for p in range(NPOS))   # accum columns per product
MIR_TOTAL = [14, 14]


def _build_nc():
    nc = bacc.Bacc(None, num_devices=NCORES, target_bir_lowering=False)

    # ---- inputs (pre-arranged in SBUF layout: [128, contiguous bytes]) ----
    # stream: 10 half-strips x [16 kchunks, 256 cols]
    xs8 = nc.declare_dram_parameter("xs8", [128, NPOS * 16 * 128], F8, isOutput=False)
    ys8 = nc.declare_dram_parameter("ys8", [128, NPOS * 16 * 128], F8, isOutput=False)
    # resident moving side (-2 x8): [16 kchunks, 512 cols]
    xm8 = nc.declare_dram_parameter("xm8", [128, 16 * ROWS], F8, isOutput=False)
    ym8 = nc.declare_dram_parameter("ym8", [128, 16 * ROWS], F8, isOutput=False)
    uabx = nc.declare_dram_parameter("uabx", [1, 2 * ROWS], F8, isOutput=False)
    uaby = nc.declare_dram_parameter("uaby", [1, 2 * ROWS], F8, isOutput=False)
    stat = nc.declare_dram_parameter("stat", [1, 256], F8, isOutput=False)
    onesf = nc.declare_dram_parameter("onesf", [128, 1], F16, isOutput=False)
    sqbx = nc.declare_dram_parameter("sqbx", [128, NPOS], F32, isOutput=False)
    sqby = nc.declare_dram_parameter("sqby", [128, NPOS], F32, isOutput=False)

    # ---- outputs ----
    csx = nc.declare_dram_parameter("csx", [128, NPOS], F32, isOutput=True)
    csy = nc.declare_dram_parameter("csy", [128, NPOS], F32, isOutput=True)
    accs = nc.declare_dram_parameter("accs", [128, 3 * NACC], F32, isOutput=True)
    adx = nc.declare_dram_parameter("adx", [128, 512], F32, isOutput=True)
    ady = nc.declare_dram_parameter("ady", [128, 512], F32, isOutput=True)
    mirs = nc.declare_dram_parameter("mirs", [128, 256], F32, isOutput=True)

    with tile.TileContext(nc) as tc:
        import contextlib

        with contextlib.ExitStack() as ctx:
            singles = ctx.enter_context(tc.tile_pool(name="singles", bufs=1))
            xstrips = ctx.enter_context(tc.tile_pool(name="xstrips", bufs=4))
            ystrips = ctx.enter_context(tc.tile_pool(name="ystrips", bufs=4))
            psum = ctx.enter_context(tc.tile_pool(name="psum", bufs=6, space="PSUM"))
            mpsum = ctx.enter_context(tc.tile_pool(name="mpsum", bufs=1, space="PSUM"))
            t32 = ctx.enter_context(tc.tile_pool(name="t32", bufs=6))
            t16 = ctx.enter_context(tc.tile_pool(name="t16", bufs=10))
            scrap = ctx.enter_context(tc.tile_pool(name="scrap", bufs=3))

            # ---- residents ----
            # big moving residents on the scalar (ACT) hwdge queue
            xm_sb = singles.tile([128, 16, ROWS], F8, name="xm_sb")
            nc.scalar.dma_start(out=xm_sb[:], in_=xm8[:, :])
            ym_sb = singles.tile([128, 16, ROWS], F8, name="ym_sb")
            nc.scalar.dma_start(out=ym_sb[:], in_=ym8[:, :])
            # small tables first on the gpsimd queue
            uabx_sb = singles.tile([1, 2, ROWS], F8, name="uabx_sb")
            nc.gpsimd.dma_start(out=uabx_sb[:], in_=uabx[:, :])
            stat_sb = singles.tile([1, 2, 128], F8, name="stat_sb")
            nc.gpsimd.dma_start(out=stat_sb[:], in_=stat[:, :])
            sqbx_sb = singles.tile([128, NPOS], F32, name="sqbx_sb")
            nc.gpsimd.dma_start(out=sqbx_sb[:], in_=sqbx[:, :])
            uaby_sb = singles.tile([1, 2, ROWS], F8, name="uaby_sb")
            nc.gpsimd.dma_start(out=uaby_sb[:], in_=uaby[:, :])
            sqby_sb = singles.tile([128, NPOS], F32, name="sqby_sb")
            nc.gpsimd.dma_start(out=sqby_sb[:], in_=sqby[:, :])
            ones_sb = singles.tile([128, 1], F16, name="ones_sb")
            nc.gpsimd.dma_start(out=ones_sb[:], in_=onesf[:, :])

            csx_sb = singles.tile([128, NPOS], F32, name="csx_sb")
            csy_sb = singles.tile([128, NPOS], F32, name="csy_sb")
            accs_sb = singles.tile([128, 3 * NACC], F32, name="accs_sb")
            mir_sb = singles.tile([128, 256], F32, name="mir_sb")
            nc.vector.memset(mir_sb[:], 0.0)

            # mirror psum groups: (m,g) -> (bank tile, base partition);
            # AP base partitions are limited to 0/32/64, so 3 groups share
            # one bank and the 4th gets its own
            mir_ps_a = mpsum.tile([128, 512], F32, name="mir_ps_a")
            mir_ps_b = mpsum.tile([128, 512], F32, name="mir_ps_b")
            mir_loc = {
                (0, 0): (mir_ps_a, 0),
                (0, 1): (mir_ps_a, 32),
                (1, 0): (mir_ps_a, 64),
                (1, 1): (mir_ps_b, 0),
            }
            mir_state = {}
            for m in range(2):
                for g in range(2):
                    mir_state[(m, g)] = [False, 0]   # started, count
            pending_mirrors = []

            def flush_mirror():
                m, g, a16t, rel = pending_mirrors.pop(0)
                stt = mir_state[(m, g)]
                stt[1] += 1
                mt, p0 = mir_loc[(m, g)]
                nc.tensor.matmul(
                    mt[p0 : p0 + 1, 0:256],
                    lhsT=ones_sb[:],
                    rhs=a16t[:, rel : rel + 256],
                    start=not stt[0],
                    stop=(stt[1] == MIR_TOTAL[g]),
                )
                stt[0] = True
                if stt[1] == MIR_TOTAL[g]:
                    po = 32 * (2 * m + g)
                    nc.scalar.activation(
                        mir_sb[po : po + 1, :], mt[p0 : p0 + 1, 0:256],
                        AF.Copy, bias=0.0, scale=1.0,
                    )

            strips = [[None] * (NPOS // 2) for _ in range(2)]

            def load_strip(m, h):
                pool, eng = (xstrips, nc.sync) if m == 0 else (ystrips, nc.gpsimd)
                src = xs8 if m == 0 else ys8
                st = pool.tile([128, 16, 256], F8, tag="st")
                eng.dma_start(out=st[:], in_=src[:, 4096 * h : 4096 * (h + 1)])
                strips[m][h] = st

            sides = (
                (xm_sb, uabx_sb, sqbx_sb, csx_sb, adx),
                (ym_sb, uaby_sb, sqby_sb, csy_sb, ady),
            )
            a16_live = [{}, {}]
            acc_col_of = {}
            _c = 0
            for pos in range(NPOS):
                acc_col_of[pos] = _c
                _c += len(_pos_ttrs(pos))

            def emit_tile(m, pos):
                m_sb, uab_sb, sqb_sb, cs_sb, ad = sides[m]
                h = pos // 2
                t = pos % 2
                if strips[m][h] is None:
                    load_strip(m, h)
                    if h + 1 < NPOS // 2 and strips[m][h + 1] is None:
                        load_strip(m, h + 1)
                strip = strips[m][h]
                c0, cw = _pos_tile(pos)
                ps = psum.tile([128, cw], F32, tag="mm")
                for kp in range(NKP):
                    nc.tensor.matmul(
                        ps[:],
                        lhsT=strip[:, 2 * kp : 2 * kp + 2, 128 * t : 128 * t + 128],
                        rhs=m_sb[:, 2 * kp : 2 * kp + 2, c0 : c0 + cw],
                        start=(kp == 0),
                        stop=False,
                        perf_mode=DR,
                    )
                nc.tensor.matmul(
                    ps[:], lhsT=stat_sb[:], rhs=uab_sb[:, :, c0 : c0 + cw],
                    start=False, stop=True, perf_mode=DR,
                )
                a32 = t32.tile([128, cw], F32, tag="a32")
                nc.scalar.activation(
                    a32[:], ps[:], AF.Sqrt,
                    bias=sqb_sb[:, pos : pos + 1], scale=1.0,
                )
                a16 = t16.tile([128, cw], F16, tag="a16")
                nc.gpsimd.tensor_scalar(
                    a16[:], a32[:], -K64, None,
                    op0=ALU.add, op1=ALU.add,
                    accum_out=cs_sb[:, pos : pos + 1],
                )
                a16_live[m][pos] = a16
                if pos < 4:
                    nc.sync.dma_start(
                        out=ad[:, 128 * pos : 128 * pos + 128],
                        in_=a32[:, 128 * pos - c0 : 128 * pos - c0 + 128],
                    )
                for g, gcol in _pos_mirrors(pos):
                    pending_mirrors.append((m, g, a16, gcol - c0))

            def emit_ttrs(pos):
                c0, _ = _pos_tile(pos)
                a16x = a16_live[0].pop(pos)
                a16y = a16_live[1][pos]
                acc_col = acc_col_of[pos]
                for seg0, segw, segwt in _pos_ttrs(pos):
                    for k, (i0t, i1t) in enumerate(
                        ((a16x, a16y), (a16x, a16x), (a16y, a16y))
                    ):
                        scr = scrap.tile([128, segw], F16, tag="scr")
                        nc.vector.tensor_tensor_reduce(
                            out=scr[:],
                            in0=i0t[:, seg0 - c0 : seg0 - c0 + segw],
                            in1=i1t[:, seg0 - c0 : seg0 - c0 + segw],
                            scale=segwt, scalar=0.0,
                            op0=ALU.mult, op1=ALU.add,
                            accum_out=accs_sb[:, k * NACC + acc_col : k * NACC + acc_col + 1],
                        )
                    acc_col += 1
                a16_live[1].pop(pos)

            # prime x strips ahead
            load_strip(0, 0)
            load_strip(0, 1)
            for i in range(NPOS + SKEW):
                if i < NPOS:
                    emit_tile(0, i)
                if i >= SKEW:
                    pos = i - SKEW
                    emit_tile(1, pos)
                    emit_ttrs(pos)
                while len(pending_mirrors) > 6:
                    flush_mirror()
            while pending_mirrors:
                flush_mirror()

            nc.sync.dma_start(out=csx[:, :], in_=csx_sb[:])
            nc.sync.dma_start(out=csy[:, :], in_=csy_sb[:])
            nc.sync.dma_start(out=accs[:, :], in_=accs_sb[:])
            nc.sync.dma_start(out=mirs[:, :], in_=mir_sb[:])

    nc.compile()
    return nc


def _get_nc():
    if "nc" not in _CACHE:
        _CACHE["nc"] = _build_nc()
    return _CACHE["nc"]


def _prep_side(F):
    x8 = np.asarray(F, dtype=np.float32).reshape(N, D).astype(f8).astype(np.float32)
    xsT = np.ascontiguousarray(x8.T).astype(f8)                 # [D, N]
    xmT = np.ascontiguousarray((-2.0 * x8).T).astype(f8)        # [D, N]
    sq = np.einsum("ij,ij->i", x8.astype(np.float64), x8.astype(np.float64))
    u = sq - 2048.0
    uA = (u / 16.0).astype(f8)
    uB = ((u - uA.astype(np.float64) * 16.0) / 2.0).astype(f8)
    sqb = (sq + 2048.0 + EB).astype(np.float32)
    return xsT, xmT, np.asarray(uA), np.asarray(uB), sqb


def _sbuf_arrange_stream(arr, c):
    """[D, N] -> [128, NPOS*16*128]: half-strip h holds k-chunks of rotated
    columns [128h, 128h+128) x [128 cols] in [k][col] order per partition."""
    start = 512 * c
    end = start + NPOS * 128
    if end <= N:
        w = arr[:, start:end]
    else:
        w = np.concatenate([arr[:, start:], arr[:, : end - N]], axis=1)
    # w: [D, NPOS*128]; per partition p: [halfstrip][k][col256], D = (k p)
    v = w.reshape(16, 128, NPOS // 2, 256)       # [k, p, hs, col]
    v = v.transpose(1, 2, 0, 3)                  # [p, hs, k, col]
    return np.ascontiguousarray(v.reshape(128, NPOS * 16 * 128))


def _sbuf_arrange_resident(arr_sl):
    """[D, ROWS] -> [128, 16*ROWS] in [k][col] order per partition."""
    v = arr_sl.reshape(16, 128, ROWS)            # [k, p, col]
    v = v.transpose(1, 0, 2)                     # [p, k, col]
    return np.ascontiguousarray(v.reshape(128, 16 * ROWS))


def _make_in_maps(featuresX, featuresY):
    xsT, xmT, uAx, uBx, sqbx = _prep_side(featuresX)
    ysT, ymT, uAy, uBy, sqby = _prep_side(featuresY)
    stat_np = np.concatenate(
        [np.full(128, 16.0, np.float32), np.full(128, 2.0, np.float32)]
    ).astype(f8).reshape(1, 256)
    ones_np = np.ones((128, 1), np.float16)

    in_maps = []
    for c in range(NCORES):
        sl = slice(c * ROWS, (c + 1) * ROWS)
        rot = [(4 * c + pos) % NJ for pos in range(NPOS)]
        sqbx_c = np.stack([sqbx[128 * g : 128 * g + 128] for g in rot], axis=1)
        sqby_c = np.stack([sqby[128 * g : 128 * g + 128] for g in rot], axis=1)
        in_maps.append(
            {
                "xs8": _sbuf_arrange_stream(xsT, c),
                "ys8": _sbuf_arrange_stream(ysT, c),
                "xm8": _sbuf_arrange_resident(xmT[:, sl]),
                "ym8": _sbuf_arrange_resident(ymT[:, sl]),
                "uabx": np.concatenate([uAx[sl], uBx[sl]]).reshape(1, 2 * ROWS),
                "uaby": np.concatenate([uAy[sl], uBy[sl]]).reshape(1, 2 * ROWS),
                "stat": stat_np,
                "onesf": ones_np,
                "sqbx": np.ascontiguousarray(sqbx_c),
                "sqby": np.ascontiguousarray(sqby_c),
            }
        )
    return in_maps


def _combine(res):
    cspx = np.zeros(N, np.float64)
    cspy = np.zeros(N, np.float64)
    P = np.zeros(3, np.float64)
    adiag_x = np.zeros(N, np.float64)
    adiag_y = np.zeros(N, np.float64)
    for c in range(NCORES):
        r = res[c]
        for pos in range(NPOS):
            gj = (4 * c + pos) % NJ
            cspx[128 * gj : 128 * gj + 128] += r["csx"][:, pos].astype(np.float64)
            cspy[128 * gj : 128 * gj + 128] += r["csy"][:, pos].astype(np.float64)
        P += r["accs"].astype(np.float64).reshape(128, 3, NACC).sum(axis=(0, 2))
        i0 = 512 * c
        mir = r["mirs"].astype(np.float64)
        cspx[i0 : i0 + 256] += mir[0]
        cspx[i0 + 256 : i0 + 512] += mir[32]
        cspy[i0 : i0 + 256] += mir[64]
        cspy[i0 + 256 : i0 + 512] += mir[96]
        for t in range(4):
            blk_x = r["adx"][:, 128 * t : 128 * t + 128]
            blk_y = r["ady"][:, 128 * t : 128 * t + 128]
            adiag_x[i0 + 128 * t : i0 + 128 * t + 128] = np.diagonal(blk_x).astype(np.float64)
            adiag_y[i0 + 128 * t : i0 + 128 * t + 128] = np.diagonal(blk_y).astype(np.float64)

    def bracket(Pv, c1p, c2p, d1, d2_):
        n = float(N)
        r1 = c1p / (n - 2)
        r2 = c2p / (n - 2)
        t1 = c1p.sum() / ((n - 1) * (n - 2)) - K64 / (n - 1)
        t2 = c2p.sum() / ((n - 1) * (n - 2)) - K64 / (n - 1)
        sv = Pv
        sv += -2.0 * (r2 @ c1p) + t2 * c1p.sum()
        sv += -2.0 * (r1 @ c2p) + t1 * c2p.sum()
        sv += 4.0 * n * (r1 @ r2)
        sv += -2.0 * n * t2 * r1.sum() - 2.0 * n * t1 * r2.sum()
        sv += n * n * t1 * t2
        A_ii = (d1 - K64) - 2.0 * r1 + t1
        B_ii = (d2_ - K64) - 2.0 * r2 + t2
        sv -= (A_ii * B_ii).sum()
        return sv / (n * (n - 3.0))

    gxy = bracket(P[0], cspx, cspy, adiag_x, adiag_y)
    gxx = bracket(P[1], cspx, cspx, adiag_x, adiag_x)
    gyy = bracket(P[2], cspy, cspy, adiag_y, adiag_y)
    loss = -gxy / np.sqrt(gxx * gyy + EPS)
    return np.array(loss, dtype=np.float32)


def kernel(featuresX: np.ndarray, featuresY: np.ndarray) -> np.ndarray:
    nc = _get_nc()
    in_maps = _make_in_maps(featuresX, featuresY)
    _CACHE["in_maps"] = in_maps
    res = run_bass_kernel_spmd(nc, in_maps, list(range(NCORES))).results
    return _combine(res)


# revision 10
# speedup vs baseline: 5.9585x; 1.0104x over previous
"""Distance-correlation loss kernel for trn2 (8 NeuronCores, SPMD).

Reference math: for F in {X, Y}: a = sqrt(relu(sq_i + sq_j - 2 F F^T) + eps),
A = a - 2*row_j + tot (row = colsum/(n-2), tot = sum/((n-1)(n-2))), zero diag;
loss = -g_xy / sqrt(g_xx * g_yy + eps), g_PQ = sum(P*Q)/(n(n-3)).

Matrix-free single-pass formulation: with a' = a - 64, every bracket sum
expands as P' (= sum a'_x a'_y and squares) plus O(n) corrections from the
shifted colsums and the measured diagonal — the device computes, per tile,
only the distance tile, its shifted colsum, and three product partials. No
second pass, no collective; the host combines per-core partials in f64.

Symmetric schedule (a is symmetric -> compute ~half): 16 virtual half-blocks
of 256 rows, 2 per core. Core c streams 20 rotated j-chunks (global chunk
(4c+pos)%32); per position the tile is
  pos 0,1:   [128,256] left  (v0 self, weight 1)
  pos 2,3:   [128,512] left w2 + right v1-self w1
  pos 4..15: [128,512] both halves w2
  pos 16,17: [128,512] left w1 (d=8 pair, both orientations), right w2
  pos 18,19: [128,256] right w1 (odd d=8 pair)
Weight-2 halves get transposed-side column sums via ones^T @ a16 matmuls
accumulated in one shared PSUM bank (groups at partitions 0/32/64/96),
emitted with a lag so PE never waits on the ACT/GPSIMD chain. Weights are
baked into the DVE tensor_tensor_reduce `scale`.

Per tile: fp8(e4m3) DoubleRow matmuls (psum = x8_strip^T (-2 x8_core)) plus
one DoubleRow pair encoding sq_i - 2048; ACT sqrt with per-partition bias
sq_j + 2048 + 0.5 (+0.5 keeps the junk diagonal positive -> no relu/NaN);
GPSIMD tensor_scalar shift a-64 -> f16 with accum_out = shifted colsum; DVE
TTR partials. Diagonal blocks sit at stream positions 0..3; their a32 column
blocks are DMA'd out and the host subtracts the exact measured diagonal.

Pipelining: host pre-arranges every fp8 array in exact SBUF layout (fully
contiguous per partition -> no sub-512B DMA penalty); strips are half-width
(1.6us each) for fine overlap; x-strips ride SP, residents the scalar queue,
small tables + y-strips gpsimd; the x stream runs SKEW positions ahead of y
so the PE starts as soon as the first x strip lands.
"""

import sys

for _p in ("/opt/trn_rl_repo",):
    if _p not in sys.path:
        sys.path.insert(0, _p)

import numpy as np
import ml_dtypes

import concourse.bass as bass
from concourse import bacc
import concourse.mybir as mybir
import concourse.tile as tile
from concourse.bass_utils import run_bass_kernel_spmd

N = 4096
D = 2048
NCORES = 8
ROWS = N // NCORES          # 512 resident rows per core
NJ = N // 128               # 32 global j-chunks
NPOS = 20                   # streamed chunk positions per core
NKP = D // 256              # 8 DoubleRow contraction pairs
SKEW = 3                    # x stream runs this many positions ahead of y
K64 = 64.0
EB = 0.5
EPS = 1e-18
F32 = mybir.dt.float32
F16 = mybir.dt.float16
F8 = mybir.dt.float8e4
AF = mybir.ActivationFunctionType
ALU = mybir.AluOpType
DR = mybir.MatmulPerfMode.DoubleRow
f8 = ml_dtypes.float8_e4m3

_CACHE = {}


def _pos_tile(pos):
    """(tile_col_start, tile_width) within the core's 512 resident columns."""
    if pos < 2:
        return 0, 256
    if pos >= 18:
        return 256, 256
    return 0, 512


def _pos_ttrs(pos):
    """(col_start, width, weight) product segments for this position."""
    if pos < 2:
        return [(0, 256, 1.0)]
    if pos < 4:
        return [(0, 256, 2.0), (256, 256, 1.0)]
    if pos < 16:
        return [(0, 512, 2.0)]
    if pos < 18:
        return [(0, 256, 1.0), (256, 256, 2.0)]
    return [(256, 256, 1.0)]


def _pos_mirrors(pos):
    """Mirror groups fed at this position: list of (group, col_start)."""
    out = []
    if 2 <= pos <= 15:
        out.append((0, 0))
    if 4 <= pos <= 17:
        out.append((1, 256))
    return out


NACC = sum(len(_pos_ttrs(p)) for p in range(NPOS))   # accum columns per product
MIR_TOTAL = [14, 14]


def _build_nc():
    nc = bacc.Bacc(None, num_devices=NCORES, target_bir_lowering=False)

    # ---- inputs (pre-arranged in SBUF layout: [128, contiguous bytes]) ----
    # stream: 10 half-strips x [16 kchunks, 256 cols]
    xs8 = nc.declare_dram_parameter("xs8", [128, NPOS * 16 * 128], F8, isOutput=False)
    ys8 = nc.declare_dram_parameter("ys8", [128, NPOS * 16 * 128], F8, isOutput=False)
    # resident moving side (-2 x8): [16 kchunks, 512 cols]
    xm8 = nc.declare_dram_parameter("xm8", [128, 16 * ROWS], F8, isOutput=False)
    ym8 = nc.declare_dram_parameter("ym8", [128, 16 * ROWS], F8, isOutput=False)
    uabx = nc.declare_dram_parameter("uabx", [1, 2 * ROWS], F8, isOutput=False)
    uaby = nc.declare_dram_parameter("uaby", [1, 2 * ROWS], F8, isOutput=False)
    stat = nc.declare_dram_parameter("stat", [1, 256], F8, isOutput=False)
    onesf = nc.declare_dram_parameter("onesf", [128, 1], F16, isOutput=False)
    sqbx = nc.declare_dram_parameter("sqbx", [128, NPOS], F32, isOutput=False)
    sqby = nc.declare_dram_parameter("sqby", [128, NPOS], F32, isOutput=False)

    # ---- outputs ----
    csx = nc.declare_dram_parameter("csx", [128, NPOS], F32, isOutput=True)
    csy = nc.declare_dram_parameter("csy", [128, NPOS], F32, isOutput=True)
    accs = nc.declare_dram_parameter("accs", [128, 3 * NACC], F32, isOutput=True)
    adx = nc.declare_dram_parameter("adx", [128, 512], F32, isOutput=True)
    ady = nc.declare_dram_parameter("ady", [128, 512], F32, isOutput=True)
    mirs = nc.declare_dram_parameter("mirs", [128, 256], F32, isOutput=True)

    with tile.TileContext(nc) as tc:
        import contextlib

        with contextlib.ExitStack() as ctx:
            singles = ctx.enter_context(tc.tile_pool(name="singles", bufs=1))
            xstrips = ctx.enter_context(tc.tile_pool(name="xstrips", bufs=4))
            ystrips = ctx.enter_context(tc.tile_pool(name="ystrips", bufs=4))
            psum = ctx.enter_context(tc.tile_pool(name="psum", bufs=6, space="PSUM"))
            mpsum = ctx.enter_context(tc.tile_pool(name="mpsum", bufs=1, space="PSUM"))
            t32 = ctx.enter_context(tc.tile_pool(name="t32", bufs=6))
            t16 = ctx.enter_context(tc.tile_pool(name="t16", bufs=10))
            scrap = ctx.enter_context(tc.tile_pool(name="scrap", bufs=3))

            # ---- residents ----
            # big moving residents on the scalar (ACT) hwdge queue, split
            # into k-halves so the first matmuls start sooner
            xm_sb = singles.tile([128, 16, ROWS], F8, name="xm_sb")
            nc.scalar.dma_start(out=xm_sb[:, 0:8, :], in_=xm8[:, : 8 * ROWS])
            nc.scalar.dma_start(out=xm_sb[:, 8:16, :], in_=xm8[:, 8 * ROWS :])
            ym_sb = singles.tile([128, 16, ROWS], F8, name="ym_sb")
            nc.scalar.dma_start(out=ym_sb[:, 0:8, :], in_=ym8[:, : 8 * ROWS])
            nc.scalar.dma_start(out=ym_sb[:, 8:16, :], in_=ym8[:, 8 * ROWS :])
            # small tables first on the gpsimd queue
            uabx_sb = singles.tile([1, 2, ROWS], F8, name="uabx_sb")
            nc.gpsimd.dma_start(out=uabx_sb[:], in_=uabx[:, :])
            stat_sb = singles.tile([1, 2, 128], F8, name="stat_sb")
            nc.gpsimd.dma_start(out=stat_sb[:], in_=stat[:, :])
            sqbx_sb = singles.tile([128, NPOS], F32, name="sqbx_sb")
            nc.gpsimd.dma_start(out=sqbx_sb[:], in_=sqbx[:, :])
            uaby_sb = singles.tile([1, 2, ROWS], F8, name="uaby_sb")
            nc.gpsimd.dma_start(out=uaby_sb[:], in_=uaby[:, :])
            sqby_sb = singles.tile([128, NPOS], F32, name="sqby_sb")
            nc.gpsimd.dma_start(out=sqby_sb[:], in_=sqby[:, :])
            ones_sb = singles.tile([128, 1], F16, name="ones_sb")
            nc.gpsimd.dma_start(out=ones_sb[:], in_=onesf[:, :])

            csx_sb = singles.tile([128, NPOS], F32, name="csx_sb")
            csy_sb = singles.tile([128, NPOS], F32, name="csy_sb")
            accs_sb = singles.tile([128, 3 * NACC], F32, name="accs_sb")
            mir_sb = singles.tile([128, 256], F32, name="mir_sb")
            nc.vector.memset(mir_sb[:], 0.0)

            # mirror psum groups: (m,g) -> (bank tile, base partition);
            # AP base partitions are limited to 0/32/64, so 3 groups share
            # one bank and the 4th gets its own
            mir_ps_a = mpsum.tile([128, 512], F32, name="mir_ps_a")
            mir_ps_b = mpsum.tile([128, 512], F32, name="mir_ps_b")
            mir_loc = {
                (0, 0): (mir_ps_a, 0),
                (0, 1): (mir_ps_a, 32),
                (1, 0): (mir_ps_a, 64),
                (1, 1): (mir_ps_b, 0),
            }
            mir_state = {}
            for m in range(2):
                for g in range(2):
                    mir_state[(m, g)] = [False, 0]   # started, count
            pending_mirrors = []

            def flush_mirror():
                m, g, a16t, rel = pending_mirrors.pop(0)
                stt = mir_state[(m, g)]
                stt[1] += 1
                mt, p0 = mir_loc[(m, g)]
                nc.tensor.matmul(
                    mt[p0 : p0 + 1, 0:256],
                    lhsT=ones_sb[:],
                    rhs=a16t[:, rel : rel + 256],
                    start=not stt[0],
                    stop=(stt[1] == MIR_TOTAL[g]),
                )
                stt[0] = True
                if stt[1] == MIR_TOTAL[g]:
                    po = 32 * (2 * m + g)
                    nc.scalar.activation(
                        mir_sb[po : po + 1, :], mt[p0 : p0 + 1, 0:256],
                        AF.Copy, bias=0.0, scale=1.0,
                    )

            strips = [[None] * (NPOS // 2) for _ in range(2)]

            def load_strip(m, h):
                pool, eng = (xstrips, nc.sync) if m == 0 else (ystrips, nc.gpsimd)
                src = xs8 if m == 0 else ys8
                st = pool.tile([128, 16, 256], F8, tag="st")
                eng.dma_start(out=st[:], in_=src[:, 4096 * h : 4096 * (h + 1)])
                strips[m][h] = st

            sides = (
                (xm_sb, uabx_sb, sqbx_sb, csx_sb, adx),
                (ym_sb, uaby_sb, sqby_sb, csy_sb, ady),
            )
            a16_live = [{}, {}]
            acc_col_of = {}
            _c = 0
            for pos in range(NPOS):
                acc_col_of[pos] = _c
                _c += len(_pos_ttrs(pos))

            def emit_tile(m, pos):
                m_sb, uab_sb, sqb_sb, cs_sb, ad = sides[m]
                h = pos // 2
                t = pos % 2
                if strips[m][h] is None:
                    load_strip(m, h)
                    if h + 1 < NPOS // 2 and strips[m][h + 1] is None:
                        load_strip(m, h + 1)
                strip = strips[m][h]
                c0, cw = _pos_tile(pos)
                ps = psum.tile([128, cw], F32, tag="mm")
                for kp in range(NKP):
                    nc.tensor.matmul(
                        ps[:],
                        lhsT=strip[:, 2 * kp : 2 * kp + 2, 128 * t : 128 * t + 128],
                        rhs=m_sb[:, 2 * kp : 2 * kp + 2, c0 : c0 + cw],
                        start=(kp == 0),
                        stop=False,
                        perf_mode=DR,
                    )
                nc.tensor.matmul(
                    ps[:], lhsT=stat_sb[:], rhs=uab_sb[:, :, c0 : c0 + cw],
                    start=False, stop=True, perf_mode=DR,
                )
                a32 = t32.tile([128, cw], F32, tag="a32")
                nc.scalar.activation(
                    a32[:], ps[:], AF.Sqrt,
                    bias=sqb_sb[:, pos : pos + 1], scale=1.0,
                )
                a16 = t16.tile([128, cw], F16, tag="a16")
                nc.gpsimd.tensor_scalar(
                    a16[:], a32[:], -K64, None,
                    op0=ALU.add, op1=ALU.add,
                    accum_out=cs_sb[:, pos : pos + 1],
                )
                a16_live[m][pos] = a16
                if pos < 4:
                    nc.sync.dma_start(
                        out=ad[:, 128 * pos : 128 * pos + 128],
                        in_=a32[:, 128 * pos - c0 : 128 * pos - c0 + 128],
                    )
                for g, gcol in _pos_mirrors(pos):
                    pending_mirrors.append((m, g, a16, gcol - c0))

            def emit_ttrs(pos):
                c0, _ = _pos_tile(pos)
                a16x = a16_live[0].pop(pos)
                a16y = a16_live[1][pos]
                acc_col = acc_col_of[pos]
                for seg0, segw, segwt in _pos_ttrs(pos):
                    for k, (i0t, i1t) in enumerate(
                        ((a16x, a16y), (a16x, a16x), (a16y, a16y))
                    ):
                        scr = scrap.tile([128, segw], F16, tag="scr")
                        nc.vector.tensor_tensor_reduce(
                            out=scr[:],
                            in0=i0t[:, seg0 - c0 : seg0 - c0 + segw],
                            in1=i1t[:, seg0 - c0 : seg0 - c0 + segw],
                            scale=segwt, scalar=0.0,
                            op0=ALU.mult, op1=ALU.add,
                            accum_out=accs_sb[:, k * NACC + acc_col : k * NACC + acc_col + 1],
                        )
                    acc_col += 1
                a16_live[1].pop(pos)

            # heavy [512] positions first; light [256] self/d8 tiles last so
            # the drain chain is short
            ORDER = list(range(2, 18)) + [0, 1, 18, 19]
            # prime x strips ahead
            load_strip(0, ORDER[0] // 2)
            load_strip(0, ORDER[1] // 2)
            for i in range(NPOS + SKEW):
                if i < NPOS:
                    emit_tile(0, ORDER[i])
                if i >= SKEW:
                    pos = ORDER[i - SKEW]
                    emit_tile(1, pos)
                    emit_ttrs(pos)
                while len(pending_mirrors) > 6:
                    flush_mirror()
            while pending_mirrors:
                flush_mirror()

            nc.sync.dma_start(out=csx[:, :], in_=csx_sb[:])
            nc.sync.dma_start(out=csy[:, :], in_=csy_sb[:])
            nc.sync.dma_start(out=accs[:, :], in_=accs_sb[:])
            nc.sync.dma_start(out=mirs[:, :], in_=mir_sb[:])

    nc.compile()
    return nc


def _get_nc():
    if "nc" not in _CACHE:
        _CACHE["nc"] = _build_nc()
    return _CACHE["nc"]


def _prep_side(F):
    x8 = np.asarray(F, dtype=np.float32).reshape(N, D).astype(f8).astype(np.float32)
    xsT = np.ascontiguousarray(x8.T).astype(f8)                 # [D, N]
    xmT = np.ascontiguousarray((-2.0 * x8).T).astype(f8)        # [D, N]
    sq = np.einsum("ij,ij->i", x8.astype(np.float64), x8.astype(np.float64))
    u = sq - 2048.0
    uA = (u / 16.0).astype(f8)
    uB = ((u - uA.astype(np.float64) * 16.0) / 2.0).astype(f8)
    sqb = (sq + 2048.0 + EB).astype(np.float32)
    return xsT, xmT, np.asarray(uA), np.asarray(uB), sqb


def _sbuf_arrange_stream(arr, c):
    """[D, N] -> [128, NPOS*16*128]: half-strip h holds k-chunks of rotated
    columns [128h, 128h+128) x [128 cols] in [k][col] order per partition."""
    start = 512 * c
    end = start + NPOS * 128
    if end <= N:
        w = arr[:, start:end]
    else:
        w = np.concatenate([arr[:, start:], arr[:, : end - N]], axis=1)
    # w: [D, NPOS*128]; per partition p: [halfstrip][k][col256], D = (k p)
    v = w.reshape(16, 128, NPOS // 2, 256)       # [k, p, hs, col]
    v = v.transpose(1, 2, 0, 3)                  # [p, hs, k, col]
    return np.ascontiguousarray(v.reshape(128, NPOS * 16 * 128))


def _sbuf_arrange_resident(arr_sl):
    """[D, ROWS] -> [128, 16*ROWS] in [k][col] order per partition."""
    v = arr_sl.reshape(16, 128, ROWS)            # [k, p, col]
    v = v.transpose(1, 0, 2)                     # [p, k, col]
    return np.ascontiguousarray(v.reshape(128, 16 * ROWS))


def _make_in_maps(featuresX, featuresY):
    xsT, xmT, uAx, uBx, sqbx = _prep_side(featuresX)
    ysT, ymT, uAy, uBy, sqby = _prep_side(featuresY)
    stat_np = np.concatenate(
        [np.full(128, 16.0, np.float32), np.full(128, 2.0, np.float32)]
    ).astype(f8).reshape(1, 256)
    ones_np = np.ones((128, 1), np.float16)

    in_maps = []
    for c in range(NCORES):
        sl = slice(c * ROWS, (c + 1) * ROWS)
        rot = [(4 * c + pos) % NJ for pos in range(NPOS)]
        sqbx_c = np.stack([sqbx[128 * g : 128 * g + 128] for g in rot], axis=1)
        sqby_c = np.stack([sqby[128 * g : 128 * g + 128] for g in rot], axis=1)
        in_maps.append(
            {
                "xs8": _sbuf_arrange_stream(xsT, c),
                "ys8": _sbuf_arrange_stream(ysT, c),
                "xm8": _sbuf_arrange_resident(xmT[:, sl]),
                "ym8": _sbuf_arrange_resident(ymT[:, sl]),
                "uabx": np.concatenate([uAx[sl], uBx[sl]]).reshape(1, 2 * ROWS),
                "uaby": np.concatenate([uAy[sl], uBy[sl]]).reshape(1, 2 * ROWS),
                "stat": stat_np,
                "onesf": ones_np,
                "sqbx": np.ascontiguousarray(sqbx_c),
                "sqby": np.ascontiguousarray(sqby_c),
            }
        )
    return in_maps


def _combine(res):
    cspx = np.zeros(N, np.float64)
    cspy = np.zeros(N, np.float64)
    P = np.zeros(3, np.float64)
    adiag_x = np.zeros(N, np.float64)
    adiag_y = np.zeros(N, np.float64)
    for c in range(NCORES):
        r = res[c]
        for pos in range(NPOS):
            gj = (4 * c + pos) % NJ
            cspx[128 * gj : 128 * gj + 128] += r["csx"][:, pos].astype(np.float64)
            cspy[128 * gj : 128 * gj + 128] += r["csy"][:, pos].astype(np.float64)
        P += r["accs"].astype(np.float64).reshape(128, 3, NACC).sum(axis=(0, 2))
        i0 = 512 * c
        mir = r["mirs"].astype(np.float64)
        cspx[i0 : i0 + 256] += mir[0]
        cspx[i0 + 256 : i0 + 512] += mir[32]
        cspy[i0 : i0 + 256] += mir[64]
        cspy[i0 + 256 : i0 + 512] += mir[96]
        for t in range(4):
            blk_x = r["adx"][:, 128 * t : 128 * t + 128]
            blk_y = r["ady"][:, 128 * t : 128 * t + 128]
            adiag_x[i0 + 128 * t : i0 + 128 * t + 128] = np.diagonal(blk_x).astype(np.float64)
            adiag_y[i0 + 128 * t : i0 + 128 * t + 128] = np.diagonal(blk_y).astype(np.float64)

    def bracket(Pv, c1p, c2p, d1, d2_):
        n = float(N)
        r1 = c1p / (n - 2)
        r2 = c2p / (n - 2)
        t1 = c1p.sum() / ((n - 1) * (n - 2)) - K64 / (n - 1)
        t2 = c2p.sum() / ((n - 1) * (n - 2)) - K64 / (n - 1)
        sv = Pv
        sv += -2.0 * (r2 @ c1p) + t2 * c1p.sum()
        sv += -2.0 * (r1 @ c2p) + t1 * c2p.sum()
        sv += 4.0 * n * (r1 @ r2)
        sv += -2.0 * n * t2 * r1.sum() - 2.0 * n * t1 * r2.sum()
        sv += n * n * t1 * t2
        A_ii = (d1 - K64) - 2.0 * r1 + t1
        B_ii = (d2_ - K64) - 2.0 * r2 + t2
        sv -= (A_ii * B_ii).sum()
        return sv / (n * (n - 3.0))

    gxy = bracket(P[0], cspx, cspy, adiag_x, adiag_y)
    gxx = bracket(P[1], cspx, cspx, adiag_x, adiag_x)
    gyy = bracket(P[2], cspy, cspy, adiag_y, adiag_y)
    loss = -gxy / np.sqrt(gxx * gyy + EPS)
    return np.array(loss, dtype=np.float32)


def kernel(featuresX: np.ndarray, featuresY: np.ndarray) -> np.ndarray:
    nc = _get_nc()
    in_maps = _make_in_maps(featuresX, featuresY)
    _CACHE["in_maps"] = in_maps
    res = run_bass_kernel_spmd(nc, in_maps, list(range(NCORES))).results
    return _combine(res)


# revision 12
# speedup vs baseline: 6.1192x; 1.0270x over previous
"""Distance-correlation loss kernel for trn2 (8 NeuronCores, SPMD).

Reference math: for F in {X, Y}: a = sqrt(relu(sq_i + sq_j - 2 F F^T) + eps),
A = a - 2*row_j + tot (row = colsum/(n-2), tot = sum/((n-1)(n-2))), zero diag;
loss = -g_xy / sqrt(g_xx * g_yy + eps), g_PQ = sum(P*Q)/(n(n-3)).

Matrix-free single-pass formulation: with a' = a - 64, every bracket sum
expands as P' (= sum a'_x a'_y and squares) plus O(n) corrections from the
shifted colsums and the measured diagonal — the device computes, per tile,
only the distance tile, its shifted colsum, and three product partials. No
second pass, no collective; the host combines per-core partials in f64.

Symmetric schedule (a is symmetric -> compute ~half): 16 virtual half-blocks
of 256 rows, 2 per core. Core c streams 20 rotated j-chunks (global chunk
(4c+pos)%32); per position the tile is
  pos 0,1:   [128,256] left  (v0 self, weight 1)
  pos 2,3:   [128,512] left w2 + right v1-self w1
  pos 4..15: [128,512] both halves w2
  pos 16,17: [128,512] left w1 (d=8 pair, both orientations), right w2
  pos 18,19: [128,256] right w1 (odd d=8 pair)
Weight-2 halves get transposed-side column sums via ones^T @ a16 matmuls
accumulated in one shared PSUM bank (groups at partitions 0/32/64/96),
emitted with a lag so PE never waits on the ACT/GPSIMD chain. Weights are
baked into the DVE tensor_tensor_reduce `scale`.

Per tile: fp8(e4m3) DoubleRow matmuls (psum = x8_strip^T (-2 x8_core)) plus
one DoubleRow pair encoding sq_i - 2048; ACT sqrt with per-partition bias
sq_j + 2048 + 0.5 (+0.5 keeps the junk diagonal positive -> no relu/NaN);
GPSIMD tensor_scalar shift a-64 -> f16 with accum_out = shifted colsum; DVE
TTR partials. Diagonal blocks sit at stream positions 0..3; their a32 column
blocks are DMA'd out and the host subtracts the exact measured diagonal.

Pipelining: host pre-arranges every fp8 array in exact SBUF layout (fully
contiguous per partition -> no sub-512B DMA penalty); strips are half-width
(1.6us each) for fine overlap; x-strips ride SP, residents the scalar queue,
small tables + y-strips gpsimd; the x stream runs SKEW positions ahead of y
so the PE starts as soon as the first x strip lands.
"""

import sys

for _p in ("/opt/trn_rl_repo",):
    if _p not in sys.path:
        sys.path.insert(0, _p)

import numpy as np
import ml_dtypes

import concourse.bass as bass
from concourse import bacc
import concourse.mybir as mybir
import concourse.tile as tile
from concourse.bass_utils import run_bass_kernel_spmd

N = 4096
D = 2048
NCORES = 8
ROWS = N // NCORES          # 512 resident rows per core
NJ = N // 128               # 32 global j-chunks
NPOS = 20                   # streamed chunk positions per core
NKP = D // 256              # 8 DoubleRow contraction pairs
SKEW = 3                    # x stream runs this many positions ahead of y
K64 = 64.0
EB = 0.5
EPS = 1e-18
F32 = mybir.dt.float32
F16 = mybir.dt.float16
F8 = mybir.dt.float8e4
AF = mybir.ActivationFunctionType
ALU = mybir.AluOpType
DR = mybir.MatmulPerfMode.DoubleRow
f8 = ml_dtypes.float8_e4m3

_CACHE = {}


def _pos_tile(pos):
    """(tile_col_start, tile_width) within the core's 512 resident columns."""
    if pos < 2:
        return 0, 256
    if pos >= 18:
        return 256, 256
    return 0, 512


def _pos_ttrs(pos):
    """(col_start, width, weight) product segments for this position."""
    if pos < 2:
        return [(0, 256, 1.0)]
    if pos < 4:
        return [(0, 256, 2.0), (256, 256, 1.0)]
    if pos < 16:
        return [(0, 512, 2.0)]
    if pos < 18:
        return [(0, 256, 1.0), (256, 256, 2.0)]
    return [(256, 256, 1.0)]


def _pos_mirrors(pos):
    """Mirror groups fed at this position: list of (group, col_start)."""
    out = []
    if 2 <= pos <= 15:
        out.append((0, 0))
    if 4 <= pos <= 17:
        out.append((1, 256))
    return out


NACC = sum(len(_pos_ttrs(p)) for p in range(NPOS))   # accum columns per product
MIR_TOTAL = [14, 14]


def _build_nc():
    nc = bacc.Bacc(None, num_devices=NCORES, target_bir_lowering=False)

    # ---- inputs (pre-arranged in SBUF layout: [128, contiguous bytes]) ----
    # stream: 10 half-strips x [16 kchunks, 256 cols]
    xs8 = nc.declare_dram_parameter("xs8", [128, NPOS * 16 * 128], F8, isOutput=False)
    ys8 = nc.declare_dram_parameter("ys8", [128, NPOS * 16 * 128], F8, isOutput=False)
    # resident moving side (-2 x8): [16 kchunks, 512 cols]
    xm8 = nc.declare_dram_parameter("xm8", [128, 16 * ROWS], F8, isOutput=False)
    ym8 = nc.declare_dram_parameter("ym8", [128, 16 * ROWS], F8, isOutput=False)
    uabx = nc.declare_dram_parameter("uabx", [1, 2 * ROWS], F8, isOutput=False)
    uaby = nc.declare_dram_parameter("uaby", [1, 2 * ROWS], F8, isOutput=False)
    stat = nc.declare_dram_parameter("stat", [1, 256], F8, isOutput=False)
    onesf = nc.declare_dram_parameter("onesf", [128, 1], F16, isOutput=False)
    sqbx = nc.declare_dram_parameter("sqbx", [128, NPOS], F32, isOutput=False)
    sqby = nc.declare_dram_parameter("sqby", [128, NPOS], F32, isOutput=False)

    # ---- outputs ----
    csx = nc.declare_dram_parameter("csx", [128, NPOS], F32, isOutput=True)
    csy = nc.declare_dram_parameter("csy", [128, NPOS], F32, isOutput=True)
    accs = nc.declare_dram_parameter("accs", [128, 3 * NACC], F32, isOutput=True)
    adx = nc.declare_dram_parameter("adx", [128, 512], F32, isOutput=True)
    ady = nc.declare_dram_parameter("ady", [128, 512], F32, isOutput=True)
    mirs = nc.declare_dram_parameter("mirs", [128, 256], F32, isOutput=True)

    with tile.TileContext(nc) as tc:
        import contextlib

        with contextlib.ExitStack() as ctx:
            singles = ctx.enter_context(tc.tile_pool(name="singles", bufs=1))
            xstrips = ctx.enter_context(tc.tile_pool(name="xstrips", bufs=4))
            ystrips = ctx.enter_context(tc.tile_pool(name="ystrips", bufs=4))
            psum = ctx.enter_context(tc.tile_pool(name="psum", bufs=6, space="PSUM"))
            mpsum = ctx.enter_context(tc.tile_pool(name="mpsum", bufs=1, space="PSUM"))
            t32 = ctx.enter_context(tc.tile_pool(name="t32", bufs=6))
            t16 = ctx.enter_context(tc.tile_pool(name="t16", bufs=10))
            scrap = ctx.enter_context(tc.tile_pool(name="scrap", bufs=3))

            # ---- residents ----
            # big moving residents on the scalar (ACT) hwdge queue, split
            # into k-halves so the first matmuls start sooner
            xm_sb = singles.tile([128, 16, ROWS], F8, name="xm_sb")
            nc.scalar.dma_start(out=xm_sb[:, 0:8, :], in_=xm8[:, : 8 * ROWS])
            nc.scalar.dma_start(out=xm_sb[:, 8:16, :], in_=xm8[:, 8 * ROWS :])
            ym_sb = singles.tile([128, 16, ROWS], F8, name="ym_sb")
            nc.scalar.dma_start(out=ym_sb[:, 0:8, :], in_=ym8[:, : 8 * ROWS])
            nc.scalar.dma_start(out=ym_sb[:, 8:16, :], in_=ym8[:, 8 * ROWS :])
            # small tables first on the gpsimd queue
            uabx_sb = singles.tile([1, 2, ROWS], F8, name="uabx_sb")
            nc.gpsimd.dma_start(out=uabx_sb[:], in_=uabx[:, :])
            stat_sb = singles.tile([1, 2, 128], F8, name="stat_sb")
            nc.gpsimd.dma_start(out=stat_sb[:], in_=stat[:, :])
            sqbx_sb = singles.tile([128, NPOS], F32, name="sqbx_sb")
            nc.gpsimd.dma_start(out=sqbx_sb[:], in_=sqbx[:, :])
            uaby_sb = singles.tile([1, 2, ROWS], F8, name="uaby_sb")
            nc.gpsimd.dma_start(out=uaby_sb[:], in_=uaby[:, :])
            sqby_sb = singles.tile([128, NPOS], F32, name="sqby_sb")
            nc.gpsimd.dma_start(out=sqby_sb[:], in_=sqby[:, :])
            ones_sb = singles.tile([128, 1], F16, name="ones_sb")
            nc.gpsimd.dma_start(out=ones_sb[:], in_=onesf[:, :])

            csx_sb = singles.tile([128, NPOS], F32, name="csx_sb")
            csy_sb = singles.tile([128, NPOS], F32, name="csy_sb")
            accs_sb = singles.tile([128, 3 * NACC], F32, name="accs_sb")
            mir_sb = singles.tile([128, 256], F32, name="mir_sb")
            nc.vector.memset(mir_sb[:], 0.0)

            # mirror psum groups: (m,g) -> (bank tile, base partition);
            # AP base partitions are limited to 0/32/64, so 3 groups share
            # one bank and the 4th gets its own
            mir_ps_a = mpsum.tile([128, 512], F32, name="mir_ps_a")
            mir_ps_b = mpsum.tile([128, 512], F32, name="mir_ps_b")
            mir_loc = {
                (0, 0): (mir_ps_a, 0),
                (0, 1): (mir_ps_a, 32),
                (1, 0): (mir_ps_a, 64),
                (1, 1): (mir_ps_b, 0),
            }
            mir_state = {}
            for m in range(2):
                for g in range(2):
                    mir_state[(m, g)] = [False, 0]   # started, count
            pending_mirrors = []

            def flush_mirror():
                m, g, a16t, rel = pending_mirrors.pop(0)
                stt = mir_state[(m, g)]
                stt[1] += 1
                mt, p0 = mir_loc[(m, g)]
                nc.tensor.matmul(
                    mt[p0 : p0 + 1, 0:256],
                    lhsT=ones_sb[:],
                    rhs=a16t[:, rel : rel + 256],
                    start=not stt[0],
                    stop=(stt[1] == MIR_TOTAL[g]),
                )
                stt[0] = True
                if stt[1] == MIR_TOTAL[g]:
                    po = 32 * (2 * m + g)
                    nc.scalar.activation(
                        mir_sb[po : po + 1, :], mt[p0 : p0 + 1, 0:256],
                        AF.Copy, bias=0.0, scale=1.0,
                    )

            strips = [[None] * (NPOS // 2) for _ in range(2)]

            def load_strip(m, h):
                pool, eng = (xstrips, nc.sync) if m == 0 else (ystrips, nc.gpsimd)
                src = xs8 if m == 0 else ys8
                st = pool.tile([128, 16, 256], F8, tag="st")
                eng.dma_start(out=st[:], in_=src[:, 4096 * h : 4096 * (h + 1)])
                strips[m][h] = st

            sides = (
                (xm_sb, uabx_sb, sqbx_sb, csx_sb, adx),
                (ym_sb, uaby_sb, sqby_sb, csy_sb, ady),
            )
            a16_live = [{}, {}]
            acc_col_of = {}
            _c = 0
            for pos in range(NPOS):
                acc_col_of[pos] = _c
                _c += len(_pos_ttrs(pos))

            def emit_tile(m, pos):
                m_sb, uab_sb, sqb_sb, cs_sb, ad = sides[m]
                h = pos // 2
                t = pos % 2
                if strips[m][h] is None:
                    load_strip(m, h)
                    if h + 1 < NPOS // 2 and strips[m][h + 1] is None:
                        load_strip(m, h + 1)
                strip = strips[m][h]
                c0, cw = _pos_tile(pos)
                ps = psum.tile([128, cw], F32, tag="mm")
                for kp in range(NKP):
                    nc.tensor.matmul(
                        ps[:],
                        lhsT=strip[:, 2 * kp : 2 * kp + 2, 128 * t : 128 * t + 128],
                        rhs=m_sb[:, 2 * kp : 2 * kp + 2, c0 : c0 + cw],
                        start=(kp == 0),
                        stop=False,
                        perf_mode=DR,
                    )
                nc.tensor.matmul(
                    ps[:], lhsT=stat_sb[:], rhs=uab_sb[:, :, c0 : c0 + cw],
                    start=False, stop=True, perf_mode=DR,
                )
                a32 = t32.tile([128, cw], F32, tag="a32")
                nc.scalar.activation(
                    a32[:], ps[:], AF.Sqrt,
                    bias=sqb_sb[:, pos : pos + 1], scale=1.0,
                )
                a16 = t16.tile([128, cw], F16, tag="a16")
                # x-shifts ride DVE (slack); y-shifts stay on gpsimd so the
                # TTRs (DVE, in-order) are never blocked behind x work
                eng = nc.vector if m == 0 else nc.gpsimd
                eng.tensor_scalar(
                    a16[:], a32[:], -K64, None,
                    op0=ALU.add, op1=ALU.add,
                    accum_out=cs_sb[:, pos : pos + 1],
                )
                a16_live[m][pos] = a16
                if pos < 4:
                    nc.sync.dma_start(
                        out=ad[:, 128 * pos : 128 * pos + 128],
                        in_=a32[:, 128 * pos - c0 : 128 * pos - c0 + 128],
                    )
                for g, gcol in _pos_mirrors(pos):
                    pending_mirrors.append((m, g, a16, gcol - c0))

            def emit_ttrs(pos):
                c0, _ = _pos_tile(pos)
                a16x = a16_live[0].pop(pos)
                a16y = a16_live[1][pos]
                acc_col = acc_col_of[pos]
                for seg0, segw, segwt in _pos_ttrs(pos):
                    for k, (i0t, i1t) in enumerate(
                        ((a16x, a16y), (a16x, a16x), (a16y, a16y))
                    ):
                        scr = scrap.tile([128, segw], F16, tag="scr")
                        nc.vector.tensor_tensor_reduce(
                            out=scr[:],
                            in0=i0t[:, seg0 - c0 : seg0 - c0 + segw],
                            in1=i1t[:, seg0 - c0 : seg0 - c0 + segw],
                            scale=segwt, scalar=0.0,
                            op0=ALU.mult, op1=ALU.add,
                            accum_out=accs_sb[:, k * NACC + acc_col : k * NACC + acc_col + 1],
                        )
                    acc_col += 1
                a16_live[1].pop(pos)

            # heavy [512] positions first; light [256] self/d8 tiles last so
            # the drain chain is short
            ORDER = list(range(2, 18)) + [0, 1, 18, 19]
            # prime the first two distinct x strips
            primed = []
            for o in ORDER:
                if o // 2 not in primed:
                    primed.append(o // 2)
                if len(primed) == 2:
                    break
            for h in primed:
                load_strip(0, h)
            for i in range(NPOS + SKEW):
                if i >= SKEW:
                    pos = ORDER[i - SKEW]
                    emit_tile(1, pos)
                    emit_ttrs(pos)
                if i < NPOS:
                    emit_tile(0, ORDER[i])
                while len(pending_mirrors) > 6:
                    flush_mirror()
            while pending_mirrors:
                flush_mirror()

            nc.sync.dma_start(out=csx[:, :], in_=csx_sb[:])
            nc.sync.dma_start(out=csy[:, :], in_=csy_sb[:])
            nc.sync.dma_start(out=accs[:, :], in_=accs_sb[:])
            nc.sync.dma_start(out=mirs[:, :], in_=mir_sb[:])

    nc.compile()
    return nc


def _get_nc():
    if "nc" not in _CACHE:
        _CACHE["nc"] = _build_nc()
    return _CACHE["nc"]


def _prep_side(F):
    x8 = np.asarray(F, dtype=np.float32).reshape(N, D).astype(f8).astype(np.float32)
    xsT = np.ascontiguousarray(x8.T).astype(f8)                 # [D, N]
    xmT = np.ascontiguousarray((-2.0 * x8).T).astype(f8)        # [D, N]
    sq = np.einsum("ij,ij->i", x8.astype(np.float64), x8.astype(np.float64))
    u = sq - 2048.0
    uA = (u / 16.0).astype(f8)
    uB = ((u - uA.astype(np.float64) * 16.0) / 2.0).astype(f8)
    sqb = (sq + 2048.0 + EB).astype(np.float32)
    return xsT, xmT, np.asarray(uA), np.asarray(uB), sqb


def _sbuf_arrange_stream(arr, c):
    """[D, N] -> [128, NPOS*16*128]: half-strip h holds k-chunks of rotated
    columns [128h, 128h+128) x [128 cols] in [k][col] order per partition."""
    start = 512 * c
    end = start + NPOS * 128
    if end <= N:
        w = arr[:, start:end]
    else:
        w = np.concatenate([arr[:, start:], arr[:, : end - N]], axis=1)
    # w: [D, NPOS*128]; per partition p: [halfstrip][k][col256], D = (k p)
    v = w.reshape(16, 128, NPOS // 2, 256)       # [k, p, hs, col]
    v = v.transpose(1, 2, 0, 3)                  # [p, hs, k, col]
    return np.ascontiguousarray(v.reshape(128, NPOS * 16 * 128))


def _sbuf_arrange_resident(arr_sl):
    """[D, ROWS] -> [128, 16*ROWS] in [k][col] order per partition."""
    v = arr_sl.reshape(16, 128, ROWS)            # [k, p, col]
    v = v.transpose(1, 0, 2)                     # [p, k, col]
    return np.ascontiguousarray(v.reshape(128, 16 * ROWS))


def _make_in_maps(featuresX, featuresY):
    xsT, xmT, uAx, uBx, sqbx = _prep_side(featuresX)
    ysT, ymT, uAy, uBy, sqby = _prep_side(featuresY)
    stat_np = np.concatenate(
        [np.full(128, 16.0, np.float32), np.full(128, 2.0, np.float32)]
    ).astype(f8).reshape(1, 256)
    ones_np = np.ones((128, 1), np.float16)

    in_maps = []
    for c in range(NCORES):
        sl = slice(c * ROWS, (c + 1) * ROWS)
        rot = [(4 * c + pos) % NJ for pos in range(NPOS)]
        sqbx_c = np.stack([sqbx[128 * g : 128 * g + 128] for g in rot], axis=1)
        sqby_c = np.stack([sqby[128 * g : 128 * g + 128] for g in rot], axis=1)
        in_maps.append(
            {
                "xs8": _sbuf_arrange_stream(xsT, c),
                "ys8": _sbuf_arrange_stream(ysT, c),
                "xm8": _sbuf_arrange_resident(xmT[:, sl]),
                "ym8": _sbuf_arrange_resident(ymT[:, sl]),
                "uabx": np.concatenate([uAx[sl], uBx[sl]]).reshape(1, 2 * ROWS),
                "uaby": np.concatenate([uAy[sl], uBy[sl]]).reshape(1, 2 * ROWS),
                "stat": stat_np,
                "onesf": ones_np,
                "sqbx": np.ascontiguousarray(sqbx_c),
                "sqby": np.ascontiguousarray(sqby_c),
            }
        )
    return in_maps


def _combine(res):
    cspx = np.zeros(N, np.float64)
    cspy = np.zeros(N, np.float64)
    P = np.zeros(3, np.float64)
    adiag_x = np.zeros(N, np.float64)
    adiag_y = np.zeros(N, np.float64)
    for c in range(NCORES):
        r = res[c]
        for pos in range(NPOS):
            gj = (4 * c + pos) % NJ
            cspx[128 * gj : 128 * gj + 128] += r["csx"][:, pos].astype(np.float64)
            cspy[128 * gj : 128 * gj + 128] += r["csy"][:, pos].astype(np.float64)
        P += r["accs"].astype(np.float64).reshape(128, 3, NACC).sum(axis=(0, 2))
        i0 = 512 * c
        mir = r["mirs"].astype(np.float64)
        cspx[i0 : i0 + 256] += mir[0]
        cspx[i0 + 256 : i0 + 512] += mir[32]
        cspy[i0 : i0 + 256] += mir[64]
        cspy[i0 + 256 : i0 + 512] += mir[96]
        for t in range(4):
            blk_x = r["adx"][:, 128 * t : 128 * t + 128]
            blk_y = r["ady"][:, 128 * t : 128 * t + 128]
            adiag_x[i0 + 128 * t : i0 + 128 * t + 128] = np.diagonal(blk_x).astype(np.float64)
            adiag_y[i0 + 128 * t : i0 + 128 * t + 128] = np.diagonal(blk_y).astype(np.float64)

    def bracket(Pv, c1p, c2p, d1, d2_):
        n = float(N)
        r1 = c1p / (n - 2)
        r2 = c2p / (n - 2)
        t1 = c1p.sum() / ((n - 1) * (n - 2)) - K64 / (n - 1)
        t2 = c2p.sum() / ((n - 1) * (n - 2)) - K64 / (n - 1)
        sv = Pv
        sv += -2.0 * (r2 @ c1p) + t2 * c1p.sum()
        sv += -2.0 * (r1 @ c2p) + t1 * c2p.sum()
        sv += 4.0 * n * (r1 @ r2)
        sv += -2.0 * n * t2 * r1.sum() - 2.0 * n * t1 * r2.sum()
        sv += n * n * t1 * t2
        A_ii = (d1 - K64) - 2.0 * r1 + t1
        B_ii = (d2_ - K64) - 2.0 * r2 + t2
        sv -= (A_ii * B_ii).sum()
        return sv / (n * (n - 3.0))

    gxy = bracket(P[0], cspx, cspy, adiag_x, adiag_y)
    gxx = bracket(P[1], cspx, cspx, adiag_x, adiag_x)
    gyy = bracket(P[2], cspy, cspy, adiag_y, adiag_y)
    loss = -gxy / np.sqrt(gxx * gyy + EPS)
    return np.array(loss, dtype=np.float32)


def kernel(featuresX: np.ndarray, featuresY: np.ndarray) -> np.ndarray:
    nc = _get_nc()
    in_maps = _make_in_maps(featuresX, featuresY)
    _CACHE["in_maps"] = in_maps
    res = run_bass_kernel_spmd(nc, in_maps, list(range(NCORES))).results
    return _combine(res)


# revision 15
# speedup vs baseline: 6.1313x; 1.0020x over previous
"""Distance-correlation loss kernel for trn2 (8 NeuronCores, SPMD).

Reference math: for F in {X, Y}: a = sqrt(relu(sq_i + sq_j - 2 F F^T) + eps),
A = a - 2*row_j + tot (row = colsum/(n-2), tot = sum/((n-1)(n-2))), zero diag;
loss = -g_xy / sqrt(g_xx * g_yy + eps), g_PQ = sum(P*Q)/(n(n-3)).

Matrix-free single-pass formulation: with a' = a - 64, every bracket sum
expands as P' (= sum a'_x a'_y and squares) plus O(n) corrections from the
shifted colsums and the measured diagonal — the device computes, per tile,
only the distance tile, its shifted colsum, and three product partials. No
second pass, no collective; the host combines per-core partials in f64.

Symmetric schedule (a is symmetric -> compute ~half): 16 virtual half-blocks
of 256 rows, 2 per core. Core c streams 20 rotated j-chunks (global chunk
(4c+pos)%32); per position the tile is
  pos 0,1:   [128,256] left  (v0 self, weight 1)
  pos 2,3:   [128,512] left w2 + right v1-self w1
  pos 4..15: [128,512] both halves w2
  pos 16,17: [128,512] left w1 (d=8 pair, both orientations), right w2
  pos 18,19: [128,256] right w1 (odd d=8 pair)
Weight-2 halves get transposed-side column sums via ones^T @ a16 matmuls
accumulated in one shared PSUM bank (groups at partitions 0/32/64/96),
emitted with a lag so PE never waits on the ACT/GPSIMD chain. Weights are
baked into the DVE tensor_tensor_reduce `scale`.

Per tile: fp8(e4m3) DoubleRow matmuls (psum = x8_strip^T (-2 x8_core)) plus
one DoubleRow pair encoding sq_i - 2048; ACT sqrt with per-partition bias
sq_j + 2048 + 0.5 (+0.5 keeps the junk diagonal positive -> no relu/NaN);
GPSIMD tensor_scalar shift a-64 -> f16 with accum_out = shifted colsum; DVE
TTR partials. Diagonal blocks sit at stream positions 0..3; their a32 column
blocks are DMA'd out and the host subtracts the exact measured diagonal.

Pipelining: host pre-arranges every fp8 array in exact SBUF layout (fully
contiguous per partition -> no sub-512B DMA penalty); strips are half-width
(1.6us each) for fine overlap; x-strips ride SP, residents the scalar queue,
small tables + y-strips gpsimd; the x stream runs SKEW positions ahead of y
so the PE starts as soon as the first x strip lands.
"""

import sys

for _p in ("/opt/trn_rl_repo",):
    if _p not in sys.path:
        sys.path.insert(0, _p)

import numpy as np
import ml_dtypes

import concourse.bass as bass
from concourse import bacc
import concourse.mybir as mybir
import concourse.tile as tile
from concourse.bass_utils import run_bass_kernel_spmd

N = 4096
D = 2048
NCORES = 8
ROWS = N // NCORES          # 512 resident rows per core
NJ = N // 128               # 32 global j-chunks
NPOS = 20                   # streamed chunk positions per core
NKP = D // 256              # 8 DoubleRow contraction pairs
SKEW = 3                    # x stream runs this many positions ahead of y
K64 = 64.0
EB = 0.5
EPS = 1e-18
F32 = mybir.dt.float32
F16 = mybir.dt.float16
F8 = mybir.dt.float8e4
AF = mybir.ActivationFunctionType
ALU = mybir.AluOpType
DR = mybir.MatmulPerfMode.DoubleRow
f8 = ml_dtypes.float8_e4m3

_CACHE = {}


def _pos_tile(pos):
    """(tile_col_start, tile_width) within the core's 512 resident columns."""
    if pos < 2:
        return 0, 256
    if pos >= 18:
        return 256, 256
    return 0, 512


def _pos_ttrs(pos):
    """(col_start, width, weight) product segments for this position."""
    if pos < 2:
        return [(0, 256, 1.0)]
    if pos < 4:
        return [(0, 256, 2.0), (256, 256, 1.0)]
    if pos < 16:
        return [(0, 512, 2.0)]
    if pos < 18:
        return [(0, 256, 1.0), (256, 256, 2.0)]
    return [(256, 256, 1.0)]


def _pos_mirrors(pos):
    """Mirror groups fed at this position: list of (group, col_start)."""
    out = []
    if 2 <= pos <= 15:
        out.append((0, 0))
    if 4 <= pos <= 17:
        out.append((1, 256))
    return out


NACC = sum(len(_pos_ttrs(p)) for p in range(NPOS))   # accum columns per product
MIR_TOTAL = [14, 14]


def _build_nc():
    nc = bacc.Bacc(None, num_devices=NCORES, target_bir_lowering=False)

    # ---- inputs (pre-arranged in SBUF layout: [128, contiguous bytes]) ----
    # stream: 10 half-strips x [16 kchunks, 256 cols]
    xs8 = nc.declare_dram_parameter("xs8", [128, NPOS * 16 * 128], F8, isOutput=False)
    ys8 = nc.declare_dram_parameter("ys8", [128, NPOS * 16 * 128], F8, isOutput=False)
    # resident moving side (-2 x8): [16 kchunks, 512 cols]
    xm8 = nc.declare_dram_parameter("xm8", [128, 16 * ROWS], F8, isOutput=False)
    ym8 = nc.declare_dram_parameter("ym8", [128, 16 * ROWS], F8, isOutput=False)
    uabx = nc.declare_dram_parameter("uabx", [1, 2 * ROWS], F8, isOutput=False)
    uaby = nc.declare_dram_parameter("uaby", [1, 2 * ROWS], F8, isOutput=False)
    stat = nc.declare_dram_parameter("stat", [1, 256], F8, isOutput=False)
    onesf = nc.declare_dram_parameter("onesf", [128, 1], F16, isOutput=False)
    sqbx = nc.declare_dram_parameter("sqbx", [128, NPOS], F32, isOutput=False)
    sqby = nc.declare_dram_parameter("sqby", [128, NPOS], F32, isOutput=False)

    # ---- outputs ----
    csx = nc.declare_dram_parameter("csx", [128, NPOS], F32, isOutput=True)
    csy = nc.declare_dram_parameter("csy", [128, NPOS], F32, isOutput=True)
    accs = nc.declare_dram_parameter("accs", [128, 3 * NACC], F32, isOutput=True)
    adx = nc.declare_dram_parameter("adx", [128, 512], F32, isOutput=True)
    ady = nc.declare_dram_parameter("ady", [128, 512], F32, isOutput=True)
    mirs = nc.declare_dram_parameter("mirs", [128, 8], F32, isOutput=True)

    with tile.TileContext(nc) as tc:
        import contextlib

        with contextlib.ExitStack() as ctx:
            singles = ctx.enter_context(tc.tile_pool(name="singles", bufs=1))
            xstrips = ctx.enter_context(tc.tile_pool(name="xstrips", bufs=4))
            ystrips = ctx.enter_context(tc.tile_pool(name="ystrips", bufs=4))
            psum = ctx.enter_context(tc.tile_pool(name="psum", bufs=6, space="PSUM"))
            mpsum = ctx.enter_context(tc.tile_pool(name="mpsum", bufs=1, space="PSUM"))
            t32 = ctx.enter_context(tc.tile_pool(name="t32", bufs=6))
            t16 = ctx.enter_context(tc.tile_pool(name="t16", bufs=10))
            scrap = ctx.enter_context(tc.tile_pool(name="scrap", bufs=3))

            # ---- residents ----
            # big moving residents on the scalar (ACT) hwdge queue, split
            # into k-halves so the first matmuls start sooner
            xm_sb = singles.tile([128, 16, ROWS], F8, name="xm_sb")
            nc.scalar.dma_start(out=xm_sb[:, 0:8, :], in_=xm8[:, : 8 * ROWS])
            nc.scalar.dma_start(out=xm_sb[:, 8:16, :], in_=xm8[:, 8 * ROWS :])
            ym_sb = singles.tile([128, 16, ROWS], F8, name="ym_sb")
            nc.scalar.dma_start(out=ym_sb[:, 0:8, :], in_=ym8[:, : 8 * ROWS])
            nc.scalar.dma_start(out=ym_sb[:, 8:16, :], in_=ym8[:, 8 * ROWS :])
            # small tables first on the gpsimd queue
            uabx_sb = singles.tile([1, 2, ROWS], F8, name="uabx_sb")
            nc.gpsimd.dma_start(out=uabx_sb[:], in_=uabx[:, :])
            stat_sb = singles.tile([1, 2, 128], F8, name="stat_sb")
            nc.gpsimd.dma_start(out=stat_sb[:], in_=stat[:, :])
            sqbx_sb = singles.tile([128, NPOS], F32, name="sqbx_sb")
            nc.gpsimd.dma_start(out=sqbx_sb[:], in_=sqbx[:, :])
            uaby_sb = singles.tile([1, 2, ROWS], F8, name="uaby_sb")
            nc.gpsimd.dma_start(out=uaby_sb[:], in_=uaby[:, :])
            sqby_sb = singles.tile([128, NPOS], F32, name="sqby_sb")
            nc.gpsimd.dma_start(out=sqby_sb[:], in_=sqby[:, :])
            ones_sb = singles.tile([128, 1], F16, name="ones_sb")
            nc.gpsimd.dma_start(out=ones_sb[:], in_=onesf[:, :])

            csx_sb = singles.tile([128, NPOS], F32, name="csx_sb")
            csy_sb = singles.tile([128, NPOS], F32, name="csy_sb")
            accs_sb = singles.tile([128, 3 * NACC], F32, name="accs_sb")
            mir_sb = singles.tile([128, 8], F32, name="mir_sb")

            # Mirror sums via the stationary trick: lhsT = a16 column chunk,
            # rhs = ones [128,1] -> out[c,0] = sum_p a16[p,c]. Output free
            # size 1 makes these matmuls ~free on the PE. All 8 accumulator
            # columns (m,g,chunk) share one PSUM bank: the first matmul's
            # start=True zeroes the whole 2KB region (emission order on the
            # in-order PE guarantees it runs first), the very last carries
            # stop=True.
            mir_ps = mpsum.tile([128, 8], F32, name="mir_ps")
            mir_emitted = [0]
            MIR_MM_TOTAL = 2 * 2 * MIR_TOTAL[0] * 2   # m * chunks * positions
            pending_mirrors = []

            def flush_mirror():
                m, g, a16t, rel = pending_mirrors.pop(0)
                for chunk in range(2):
                    col = 4 * m + 2 * g + chunk
                    mir_emitted[0] += 1
                    nc.tensor.matmul(
                        mir_ps[:, col : col + 1],
                        lhsT=a16t[:, rel + 128 * chunk : rel + 128 * chunk + 128],
                        rhs=ones_sb[:],
                        start=(mir_emitted[0] == 1),
                        stop=(mir_emitted[0] == MIR_MM_TOTAL),
                    )
                if mir_emitted[0] == MIR_MM_TOTAL:
                    nc.scalar.activation(
                        mir_sb[:], mir_ps[:],
                        AF.Copy, bias=0.0, scale=1.0,
                    )

            strips = [[None] * (NPOS // 2) for _ in range(2)]

            def load_strip(m, h):
                pool, eng = (xstrips, nc.sync) if m == 0 else (ystrips, nc.gpsimd)
                src = xs8 if m == 0 else ys8
                st = pool.tile([128, 16, 256], F8, tag="st")
                eng.dma_start(out=st[:], in_=src[:, 4096 * h : 4096 * (h + 1)])
                strips[m][h] = st

            sides = (
                (xm_sb, uabx_sb, sqbx_sb, csx_sb, adx),
                (ym_sb, uaby_sb, sqby_sb, csy_sb, ady),
            )
            a16_live = [{}, {}]
            acc_col_of = {}
            _c = 0
            for pos in range(NPOS):
                acc_col_of[pos] = _c
                _c += len(_pos_ttrs(pos))

            def emit_tile(m, pos):
                m_sb, uab_sb, sqb_sb, cs_sb, ad = sides[m]
                h = pos // 2
                t = pos % 2
                if strips[m][h] is None:
                    load_strip(m, h)
                    if h + 1 < NPOS // 2 and strips[m][h + 1] is None:
                        load_strip(m, h + 1)
                strip = strips[m][h]
                c0, cw = _pos_tile(pos)
                ps = psum.tile([128, cw], F32, tag="mm")
                for kp in range(NKP):
                    nc.tensor.matmul(
                        ps[:],
                        lhsT=strip[:, 2 * kp : 2 * kp + 2, 128 * t : 128 * t + 128],
                        rhs=m_sb[:, 2 * kp : 2 * kp + 2, c0 : c0 + cw],
                        start=(kp == 0),
                        stop=False,
                        perf_mode=DR,
                    )
                nc.tensor.matmul(
                    ps[:], lhsT=stat_sb[:], rhs=uab_sb[:, :, c0 : c0 + cw],
                    start=False, stop=True, perf_mode=DR,
                )
                a32 = t32.tile([128, cw], F32, tag="a32")
                nc.scalar.activation(
                    a32[:], ps[:], AF.Sqrt,
                    bias=sqb_sb[:, pos : pos + 1], scale=1.0,
                )
                a16 = t16.tile([128, cw], F16, tag="a16")
                # x-shifts ride DVE (slack); y-shifts stay on gpsimd so the
                # TTRs (DVE, in-order) are never blocked behind x work
                eng = nc.vector if m == 0 else nc.gpsimd
                eng.tensor_scalar(
                    a16[:], a32[:], -K64, None,
                    op0=ALU.add, op1=ALU.add,
                    accum_out=cs_sb[:, pos : pos + 1],
                )
                a16_live[m][pos] = a16
                if pos < 4:
                    nc.sync.dma_start(
                        out=ad[:, 128 * pos : 128 * pos + 128],
                        in_=a32[:, 128 * pos - c0 : 128 * pos - c0 + 128],
                    )
                for g, gcol in _pos_mirrors(pos):
                    pending_mirrors.append((m, g, a16, gcol - c0))

            def emit_ttrs(pos):
                c0, _ = _pos_tile(pos)
                a16x = a16_live[0].pop(pos)
                a16y = a16_live[1][pos]
                acc_col = acc_col_of[pos]
                for seg0, segw, segwt in _pos_ttrs(pos):
                    for k, (i0t, i1t) in enumerate(
                        ((a16x, a16y), (a16x, a16x), (a16y, a16y))
                    ):
                        scr = scrap.tile([128, segw], F16, tag="scr")
                        nc.vector.tensor_tensor_reduce(
                            out=scr[:],
                            in0=i0t[:, seg0 - c0 : seg0 - c0 + segw],
                            in1=i1t[:, seg0 - c0 : seg0 - c0 + segw],
                            scale=segwt, scalar=0.0,
                            op0=ALU.mult, op1=ALU.add,
                            accum_out=accs_sb[:, k * NACC + acc_col : k * NACC + acc_col + 1],
                        )
                    acc_col += 1
                a16_live[1].pop(pos)

            # heavy [512] positions first; light [256] self/d8 tiles last so
            # the drain chain is short
            ORDER = list(range(2, 18)) + [0, 1, 18, 19]
            # prime the first two distinct x strips
            primed = []
            for o in ORDER:
                if o // 2 not in primed:
                    primed.append(o // 2)
                if len(primed) == 2:
                    break
            for h in primed:
                load_strip(0, h)
            for i in range(NPOS + SKEW):
                if i >= SKEW:
                    pos = ORDER[i - SKEW]
                    emit_tile(1, pos)
                    emit_ttrs(pos)
                if i < NPOS:
                    emit_tile(0, ORDER[i])
                while len(pending_mirrors) > 6:
                    flush_mirror()
            while pending_mirrors:
                flush_mirror()

            nc.sync.dma_start(out=csx[:, :], in_=csx_sb[:])
            nc.sync.dma_start(out=csy[:, :], in_=csy_sb[:])
            nc.sync.dma_start(out=accs[:, :], in_=accs_sb[:])
            nc.sync.dma_start(out=mirs[:, :], in_=mir_sb[:])

    nc.compile()
    return nc


def _get_nc():
    if "nc" not in _CACHE:
        _CACHE["nc"] = _build_nc()
    return _CACHE["nc"]


def _prep_side(F):
    x8 = np.asarray(F, dtype=np.float32).reshape(N, D).astype(f8).astype(np.float32)
    xsT = np.ascontiguousarray(x8.T).astype(f8)                 # [D, N]
    xmT = np.ascontiguousarray((-2.0 * x8).T).astype(f8)        # [D, N]
    sq = np.einsum("ij,ij->i", x8.astype(np.float64), x8.astype(np.float64))
    u = sq - 2048.0
    uA = (u / 16.0).astype(f8)
    uB = ((u - uA.astype(np.float64) * 16.0) / 2.0).astype(f8)
    sqb = (sq + 2048.0 + EB).astype(np.float32)
    return xsT, xmT, np.asarray(uA), np.asarray(uB), sqb


def _sbuf_arrange_stream(arr, c):
    """[D, N] -> [128, NPOS*16*128]: half-strip h holds k-chunks of rotated
    columns [128h, 128h+128) x [128 cols] in [k][col] order per partition."""
    start = 512 * c
    end = start + NPOS * 128
    if end <= N:
        w = arr[:, start:end]
    else:
        w = np.concatenate([arr[:, start:], arr[:, : end - N]], axis=1)
    # w: [D, NPOS*128]; per partition p: [halfstrip][k][col256], D = (k p)
    v = w.reshape(16, 128, NPOS // 2, 256)       # [k, p, hs, col]
    v = v.transpose(1, 2, 0, 3)                  # [p, hs, k, col]
    return np.ascontiguousarray(v.reshape(128, NPOS * 16 * 128))


def _sbuf_arrange_resident(arr_sl):
    """[D, ROWS] -> [128, 16*ROWS] in [k][col] order per partition."""
    v = arr_sl.reshape(16, 128, ROWS)            # [k, p, col]
    v = v.transpose(1, 0, 2)                     # [p, k, col]
    return np.ascontiguousarray(v.reshape(128, 16 * ROWS))


def _make_in_maps(featuresX, featuresY):
    xsT, xmT, uAx, uBx, sqbx = _prep_side(featuresX)
    ysT, ymT, uAy, uBy, sqby = _prep_side(featuresY)
    stat_np = np.concatenate(
        [np.full(128, 16.0, np.float32), np.full(128, 2.0, np.float32)]
    ).astype(f8).reshape(1, 256)
    ones_np = np.ones((128, 1), np.float16)

    in_maps = []
    for c in range(NCORES):
        sl = slice(c * ROWS, (c + 1) * ROWS)
        rot = [(4 * c + pos) % NJ for pos in range(NPOS)]
        sqbx_c = np.stack([sqbx[128 * g : 128 * g + 128] for g in rot], axis=1)
        sqby_c = np.stack([sqby[128 * g : 128 * g + 128] for g in rot], axis=1)
        in_maps.append(
            {
                "xs8": _sbuf_arrange_stream(xsT, c),
                "ys8": _sbuf_arrange_stream(ysT, c),
                "xm8": _sbuf_arrange_resident(xmT[:, sl]),
                "ym8": _sbuf_arrange_resident(ymT[:, sl]),
                "uabx": np.concatenate([uAx[sl], uBx[sl]]).reshape(1, 2 * ROWS),
                "uaby": np.concatenate([uAy[sl], uBy[sl]]).reshape(1, 2 * ROWS),
                "stat": stat_np,
                "onesf": ones_np,
                "sqbx": np.ascontiguousarray(sqbx_c),
                "sqby": np.ascontiguousarray(sqby_c),
            }
        )
    return in_maps


def _combine(res):
    cspx = np.zeros(N, np.float64)
    cspy = np.zeros(N, np.float64)
    P = np.zeros(3, np.float64)
    adiag_x = np.zeros(N, np.float64)
    adiag_y = np.zeros(N, np.float64)
    for c in range(NCORES):
        r = res[c]
        for pos in range(NPOS):
            gj = (4 * c + pos) % NJ
            cspx[128 * gj : 128 * gj + 128] += r["csx"][:, pos].astype(np.float64)
            cspy[128 * gj : 128 * gj + 128] += r["csy"][:, pos].astype(np.float64)
        P += r["accs"].astype(np.float64).reshape(128, 3, NACC).sum(axis=(0, 2))
        i0 = 512 * c
        mir = r["mirs"].astype(np.float64)
        cspx[i0 : i0 + 256] += mir[0]
        cspx[i0 + 256 : i0 + 512] += mir[32]
        cspy[i0 : i0 + 256] += mir[64]
        cspy[i0 + 256 : i0 + 512] += mir[96]
        for t in range(4):
            blk_x = r["adx"][:, 128 * t : 128 * t + 128]
            blk_y = r["ady"][:, 128 * t : 128 * t + 128]
            adiag_x[i0 + 128 * t : i0 + 128 * t + 128] = np.diagonal(blk_x).astype(np.float64)
            adiag_y[i0 + 128 * t : i0 + 128 * t + 128] = np.diagonal(blk_y).astype(np.float64)

    def bracket(Pv, c1p, c2p, d1, d2_):
        n = float(N)
        r1 = c1p / (n - 2)
        r2 = c2p / (n - 2)
        t1 = c1p.sum() / ((n - 1) * (n - 2)) - K64 / (n - 1)
        t2 = c2p.sum() / ((n - 1) * (n - 2)) - K64 / (n - 1)
        sv = Pv
        sv += -2.0 * (r2 @ c1p) + t2 * c1p.sum()
        sv += -2.0 * (r1 @ c2p) + t1 * c2p.sum()
        sv += 4.0 * n * (r1 @ r2)
        sv += -2.0 * n * t2 * r1.sum() - 2.0 * n * t1 * r2.sum()
        sv += n * n * t1 * t2
        A_ii = (d1 - K64) - 2.0 * r1 + t1
        B_ii = (d2_ - K64) - 2.0 * r2 + t2
        sv -= (A_ii * B_ii).sum()
        return sv / (n * (n - 3.0))

    gxy = bracket(P[0], cspx, cspy, adiag_x, adiag_y)
    gxx = bracket(P[1], cspx, cspx, adiag_x, adiag_x)
    gyy = bracket(P[2], cspy, cspy, adiag_y, adiag_y)
    loss = -gxy / np.sqrt(gxx * gyy + EPS)
    return np.array(loss, dtype=np.float32)


def kernel(featuresX: np.ndarray, featuresY: np.ndarray) -> np.ndarray:
    nc = _get_nc()
    in_maps = _make_in_maps(featuresX, featuresY)
    _CACHE["in_maps"] = in_maps
    res = run_bass_kernel_spmd(nc, in_maps, list(range(NCORES))).results
    return _combine(res)
